# revision 2
# baseline (speedup 1.0000x reference)
"""CFSDP (density-peaks clustering) on 8 Trainium2 NeuronCores.

Pipeline (N=8192 points, D=64, row-sharded 1024 rows/core):
  d2(i,j) = ||xi-xj||^2 via one K=68 augmented matmul per tile:
      u_i = (-2*x_i, sqh_i, sql_i, 1, 1),  v_j = (x_j, 1, 1, sqh_j, sql_j)
  All O(N^2) math runs on squared distances (sqrt is monotone, so order
  stats / argmin / percentile commute with it):
    L12 launch: tanh step-counts for 4 thresholds around the predicted
        2%-quantile (tanh lives in the same ACT table set as exp, so no
        mid-kernel table reload) -> on-device dc^2 interpolation -> rho
        via ACT Exp + accumulate.  Input DMA split across the sync and
        scalar HWDGE queues so count inputs land first.
    host: stable-sort rows by rho desc; per-row prefix cutoffs.
    L3 launch: delta^2 = min d2 over the sorted prefix.  Full col-groups:
        ACT copies psum->bf16, DVE min-reduces at 2 elem/cycle.  Boundary
        window: one DVE scalar_tensor_tensor max(1e6*iota - cutsc, d2)
        puts a >=5e5 marker on masked cols, then min-reduce.
  Host finishes: delta fallback (row max) for top-density rows, nhd argmin
  (lazy, only for non-center points), center ranks, label propagation scan.
"""

import os
import numpy as np

N = 8192
D = 64
NCORES = 8
ROWS = N // NCORES          # 1024 rows per core
P = 128                     # partitions
RB = ROWS // P              # 8 row-blocks per core
FD = 2048                   # free-dim group (4 PSUM banks)
G = N // FD                 # 4 col-groups per row
K = D + 4                   # 68 (augmented contraction dim, sq split hi+lo)
MM_N = 512                  # cols per matmul (one PSUM bank output)
MM_PER_G = FD // MM_N       # 4

NT = 4                      # percentile-count thresholds
L1_W = 1024                 # cols counted per threshold
DC2_CENTER = 86.2           # chi^2_64-predicted 2%-quantile of d2 (randn data)
DC2_GRID = (DC2_CENTER * (1.0 + (np.arange(NT) - (NT - 1) / 2) * 0.023)).astype(
    np.float64
)                           # +-3.5% bracket, 2.3% spacing
SIG_ALPHA = 2.0e4           # sigmoid-equivalent step sharpness
PCT = 2.0
WW = 1024                   # L3 boundary mask window width
NCOL = G + 1                # L3 output cols per block (G group slots + window)
MASK_SCALE = 1.0e6          # L3 iota penalty scale; masked cols >= 5e5
EMPTY_SENTINEL = 1.0e5      # accum >= this => empty prefix window

# threshold b is counted on group (m, g) of every core (1/16 of the matrix
# per threshold => ~4.2M samples each; different rows+cols per threshold)
L1_GROUPS = [(b % RB, 1 + b % (G - 1)) for b in range(NT)]  # g>0: diag-free
DC2_STEP = float(DC2_CENTER * 0.023)
M_TOT = float(N) * float(N)
K_POS = PCT / 100.0 * (M_TOT - 1.0)
P_OFF = (K_POS - N) / (M_TOT - N)      # diag-free target CDF
CSTAR = float(P_OFF * P * L1_W)        # target count over the device sample
CSTAR2 = 2.0 * CSTAR                   # tanh-units target (accum = 2c - W)

_programs: dict = {}


def _build_l12():
    """Merged count + rho launch: dc^2 is computed ON DEVICE.

    Every core counts the SAME sample (rows 0..1023 via the shared `uvc`
    lhsT, diag-free col groups), so each core independently derives an
    identical dc^2 - no collectives.  Counts use ACT Tanh as the step
    function (same activation-table set as Exp -> single table load);
    per-partition accum = 2*count - L1_W, handled by transformed host
    constants (dvec' = 2*dvec - P*L1_W, CSTAR2 = 2*CSTAR).  The CDF
    interpolation runs as tiny [1,NT] vector ops; a PE ones-matmul does
    the partition reduction and a K=1 fp32 matmul broadcasts -1/dc^2 to
    all partitions for the rho phase.
    """
    import concourse.mybir as mybir
    import concourse.tile as tile
    from concourse import bacc

    f32 = mybir.dt.float32
    nc = bacc.Bacc("TRN2", debug=False, enable_asserts=False)
    bf16 = mybir.dt.bfloat16
    uv_d = nc.dram_tensor("uv", [K, ROWS + N], bf16, kind="ExternalInput")
    uvc_d = nc.dram_tensor("uvc", [K, ROWS], bf16, kind="ExternalInput")
    thr_d = nc.dram_tensor("thr", [P, NT], f32, kind="ExternalInput")
    tvec_d = nc.dram_tensor("tvec", [1, NT], f32, kind="ExternalInput")
    dvec_d = nc.dram_tensor("dvec", [1, NT], f32, kind="ExternalInput")
    cnt_d = nc.dram_tensor("counts", [P, NT], f32, kind="ExternalOutput")
    rho_d = nc.dram_tensor("rho", [P, RB], f32, kind="ExternalOutput")

    with tile.TileContext(nc) as tc:
        with (
            tc.tile_pool(name="inp", bufs=1) as inp,
            tc.tile_pool(name="stat", bufs=1) as stat,
            tc.tile_pool(name="trash", bufs=2) as trash_p,
            tc.tile_pool(name="psum", bufs=2, space="PSUM") as psum_p,
        ):
            uv_sb = inp.tile([K, ROWS + N], bf16)
            uvc_sb = inp.tile([K, ROWS], bf16)
            # count inputs first, split across the two HWDGE queues:
            #   sync:   uvc, w1, g0-full, then g1/g2/g3 remainders
            #   scalar: w2, w3, U-rows
            W0 = ROWS  # VT base
            nc.sync.dma_start(out=uvc_sb[:], in_=uvc_d[:])
            _w = lambda g: (W0 + g * FD, W0 + g * FD + L1_W)
            a1, b1 = _w(1)
            nc.sync.dma_start(out=uv_sb[:, a1:b1], in_=uv_d[:, a1:b1])
            a2, b2 = _w(2)
            nc.scalar.dma_start(out=uv_sb[:, a2:b2], in_=uv_d[:, a2:b2])
            a3, b3 = _w(3)
            nc.scalar.dma_start(out=uv_sb[:, a3:b3], in_=uv_d[:, a3:b3])
            nc.scalar.dma_start(out=uv_sb[:, 0:ROWS], in_=uv_d[:, 0:ROWS])
            nc.sync.dma_start(out=uv_sb[:, W0:W0 + FD], in_=uv_d[:, W0:W0 + FD])
            for _g in (1, 2, 3):
                _a = W0 + _g * FD + L1_W
                _b = W0 + (_g + 1) * FD
                eng = nc.sync if _g != 2 else nc.scalar
                eng.dma_start(out=uv_sb[:, _a:_b], in_=uv_d[:, _a:_b])
            thr_sb = inp.tile([P, NT], f32)
            nc.gpsimd.dma_start(out=thr_sb[:], in_=thr_d[:])
            tdv_sb = inp.tile([1, 2 * NT], f32)
            nc.gpsimd.dma_start(out=tdv_sb[:, 0:NT], in_=tvec_d[:])
            nc.gpsimd.dma_start(out=tdv_sb[:, NT:2 * NT], in_=dvec_d[:])
            cnts = stat.tile([P, NT], f32)
            warmact = stat.tile([P, 1], f32)
            nc.scalar.activation(
                warmact[:], thr_sb[:, 0:1],
                mybir.ActivationFunctionType.Tanh, bias=0.0, scale=1.0,
            )

            # ---- phase 1: counts over the shared sample -----------------
            for b, (m, g) in enumerate(L1_GROUPS):
                psum = psum_p.tile([P, FD], f32, tag="psum")
                for j in range(L1_W // MM_N):
                    nc.tensor.matmul(
                        psum[:, j * MM_N:(j + 1) * MM_N],
                        uvc_sb[:, m * P:(m + 1) * P],
                        uv_sb[:, W0 + g * FD + j * MM_N: W0 + g * FD + (j + 1) * MM_N],
                        start=True,
                        stop=True,
                    )
                t = trash_p.tile([P, L1_W], f32, tag="cntrash")
                nc.scalar.activation(
                    t[:],
                    psum[:, 0:L1_W],
                    mybir.ActivationFunctionType.Tanh,
                    bias=thr_sb[:, b:b + 1],
                    scale=float(-0.5 * SIG_ALPHA),
                    accum_out=cnts[:, b:b + 1],
                )
            nc.gpsimd.dma_start(out=cnt_d[:], in_=cnts[:])

            # ---- phase 2: dc^2 from counts (identical on every core) ----
            ones_col = stat.tile([P, 1], f32)
            nc.vector.memset(ones_col[:], 1.0)
            ps_tot = psum_p.tile([1, NT], f32, tag="psum")
            nc.tensor.matmul(ps_tot[:], ones_col[:], cnts[:], start=True, stop=True)
            w = stat.tile([1, 8 * NT], f32)  # scratch lanes along free dim
            q = w[:, 0:NT]
            nc.vector.tensor_tensor(
                out=q, in0=ps_tot[:], in1=tdv_sb[:, NT:2 * NT],
                op=mybir.AluOpType.subtract,
            )
            NB_ = NT - 1
            a_ = w[:, NT:NT + NB_]
            nc.vector.tensor_scalar(
                out=a_, in0=q[:, 0:NB_], scalar1=CSTAR2, scalar2=None,
                op0=mybir.AluOpType.is_le,
            )
            b_ = w[:, 2 * NT:2 * NT + NB_]
            nc.vector.tensor_scalar(
                out=b_, in0=q[:, 1:NT], scalar1=CSTAR2, scalar2=None,
                op0=mybir.AluOpType.is_gt,
            )
            sel = w[:, 3 * NT:3 * NT + NB_]
            nc.vector.tensor_tensor(out=sel, in0=a_, in1=b_, op=mybir.AluOpType.mult)
            den = w[:, 4 * NT:4 * NT + NB_]
            nc.vector.tensor_tensor(
                out=den, in0=q[:, 1:NT], in1=q[:, 0:NB_],
                op=mybir.AluOpType.subtract,
            )
            rec = w[:, 5 * NT:5 * NT + NB_]
            nc.vector.reciprocal(rec, den)
            num = w[:, 6 * NT:6 * NT + NB_]
            nc.vector.tensor_scalar(
                out=num, in0=q[:, 0:NB_], scalar1=-1.0, scalar2=CSTAR2,
                op0=mybir.AluOpType.mult, op1=mybir.AluOpType.add,
            )
            fr = w[:, 7 * NT:7 * NT + NB_]
            nc.vector.tensor_tensor(out=fr, in0=num, in1=rec, op=mybir.AluOpType.mult)
            nc.vector.tensor_scalar(
                out=fr, in0=fr, scalar1=float(DC2_STEP), scalar2=None,
                op0=mybir.AluOpType.mult,
            )
            nc.vector.tensor_tensor(
                out=fr, in0=fr, in1=tdv_sb[:, 0:NB_], op=mybir.AluOpType.add
            )
            nc.vector.tensor_tensor(out=fr, in0=fr, in1=sel, op=mybir.AluOpType.mult)
            sc = stat.tile([1, 4], f32)
            nc.vector.tensor_reduce(
                sc[:, 0:1], fr[:], axis=mybir.AxisListType.X, op=mybir.AluOpType.add
            )
            nc.vector.tensor_reduce(
                sc[:, 1:2], sel[:], axis=mybir.AxisListType.X, op=mybir.AluOpType.add
            )
            # guard: if no bracket, fall back to the grid center
            nc.vector.tensor_scalar(
                out=sc[:, 2:3], in0=sc[:, 1:2], scalar1=float(-DC2_CENTER),
                scalar2=float(DC2_CENTER), op0=mybir.AluOpType.mult,
                op1=mybir.AluOpType.add,
            )
            nc.vector.tensor_tensor(
                out=sc[:, 0:1], in0=sc[:, 0:1], in1=sc[:, 2:3],
                op=mybir.AluOpType.add,
            )
            nc.vector.reciprocal(sc[:, 3:4], sc[:, 0:1])
            nc.vector.tensor_scalar(
                out=sc[:, 3:4], in0=sc[:, 3:4], scalar1=-1.0, scalar2=None,
                op0=mybir.AluOpType.mult,
            )
            ones_row = stat.tile([1, P], f32)
            nc.vector.memset(ones_row[:], 1.0)
            ps_b = psum_p.tile([P, 1], f32, tag="psum")
            nc.tensor.matmul(ps_b[:], ones_row[:], sc[:, 3:4], start=True, stop=True)
            scl_sb = stat.tile([P, 1], f32)
            nc.vector.tensor_copy(scl_sb[:], ps_b[:])

            # ---- phase 3: rho ------------------------------------------
            parts = stat.tile([P, RB * G], f32)
            rho_sb = stat.tile([P, RB], f32)
            for m in range(RB):
                for g in range(G):
                    psum = psum_p.tile([P, FD], f32, tag="psum")
                    for j in range(MM_PER_G):
                        nc.tensor.matmul(
                            psum[:, j * MM_N:(j + 1) * MM_N],
                            uv_sb[:, m * P:(m + 1) * P],
                            uv_sb[:, W0 + g * FD + j * MM_N: W0 + g * FD + (j + 1) * MM_N],
                            start=True,
                            stop=True,
                        )
                    t = trash_p.tile([P, FD], f32, tag="trash")
                    q2 = m * G + g
                    nc.scalar.activation(
                        t[:],
                        psum[:],
                        mybir.ActivationFunctionType.Exp,
                        bias=0.0,
                        scale=scl_sb[:, 0:1],
                        accum_out=parts[:, q2:q2 + 1],
                    )
                nc.vector.tensor_reduce(
                    rho_sb[:, m:m + 1],
                    parts[:, m * G:(m + 1) * G],
                    axis=mybir.AxisListType.X,
                    op=mybir.AluOpType.add,
                )
            nc.sync.dma_start(out=rho_d[:], in_=rho_sb[:])
    nc.compile()
    return nc


def _build_l3():
    """Delta pass on rho-sorted data (round-robin block interleaving).

    Core c holds sorted row-blocks b = 8m + c (m = 0..7).  For local block
    m: boundary col-group gb = m//2, window base w_lo = 1024*(m%2).
    Structure per block:
      groups g < gb:  ACT copies psum -> bf16, DVE min-reduce (2 el/cyc).
      boundary group: one scalar_tensor_tensor
          out = max(MASK_SCALE*(iota - w_hi) - cutsc, d2)
        marks cols >= the per-row cutoff with values >= 5e5, then a
        min-reduce.  Odd m reduces the whole [P,2048] group (prefix cols
        get iota < 0 so they pass through); even m only [P,1024].
      columns beyond the window are never matmul'd.
    Ties that push a row's cutoff below its block's boundary window are
    patched exactly on the host (straddle_fix), as are empty prefixes
    (marker min >= EMPTY_SENTINEL).
    """
    import concourse.mybir as mybir
    import concourse.tile as tile
    from concourse import bacc

    f32 = mybir.dt.float32
    nc = bacc.Bacc("TRN2", debug=False, enable_asserts=False)
    bf16 = mybir.dt.bfloat16
    uv_d = nc.dram_tensor("uv", [K, ROWS + N], bf16, kind="ExternalInput")
    io_d = nc.dram_tensor("io", [P, FD], f32, kind="ExternalInput")
    cut_d = nc.dram_tensor("cut", [P, RB], f32, kind="ExternalInput")
    dmin_d = nc.dram_tensor("dmin", [P, RB * NCOL], f32, kind="ExternalOutput")

    with tile.TileContext(nc) as tc:
        with (
            tc.tile_pool(name="inp", bufs=1) as inp,
            tc.tile_pool(name="stat", bufs=1) as stat,
            tc.tile_pool(name="tbf", bufs=3) as tbf_p,
            tc.tile_pool(name="wf", bufs=2) as wf_p,
            tc.tile_pool(name="wb", bufs=2) as wb_p,
            tc.tile_pool(name="psum", bufs=2, space="PSUM") as psum_p,
        ):
            uv_sb = inp.tile([K, ROWS + N], bf16)
            nc.sync.dma_start(out=uv_sb[:, 0:ROWS], in_=uv_d[:, 0:ROWS])
            for _g in range(G):
                _a = ROWS + _g * FD
                eng = nc.sync if _g < 2 else nc.scalar
                eng.dma_start(out=uv_sb[:, _a:_a + FD], in_=uv_d[:, _a:_a + FD])
            io_sb = inp.tile([P, FD], f32)
            nc.gpsimd.dma_start(out=io_sb[:], in_=io_d[:])
            cut_sb = inp.tile([P, RB], f32)
            nc.gpsimd.dma_start(out=cut_sb[:], in_=cut_d[:])
            dmin_sb = stat.tile([P, RB * NCOL], f32)

            for m in range(RB):
                gb = m // 2
                w_lo = WW * (m % 2)
                bw = w_lo + WW          # matmul'd cols in the boundary group
                for g in range(gb + 1):
                    ncols = FD if g < gb else bw
                    psum = psum_p.tile([P, FD], f32, tag="psum")
                    for j in range(ncols // MM_N):
                        nc.tensor.matmul(
                            psum[:, j * MM_N:(j + 1) * MM_N],
                            uv_sb[:, m * P:(m + 1) * P],
                            uv_sb[:, ROWS + g * FD + j * MM_N: ROWS + g * FD + (j + 1) * MM_N],
                            start=True,
                            stop=True,
                        )
                    q = m * NCOL + g
                    if g < gb:
                        t = tbf_p.tile([P, FD], bf16, tag="tbf")
                        nc.scalar.activation(
                            t[:], psum[:], mybir.ActivationFunctionType.Copy,
                            bias=0.0, scale=1.0,
                        )
                        nc.vector.tensor_reduce(
                            dmin_sb[:, q:q + 1],
                            t[:],
                            axis=mybir.AxisListType.X,
                            op=mybir.AluOpType.min,
                        )
                    elif w_lo == 0:
                        wt = wf_p.tile([P, WW], f32, tag="wf")
                        nc.vector.scalar_tensor_tensor(
                            out=wt[:],
                            in0=io_sb[:, WW:FD],
                            scalar=cut_sb[:, m:m + 1],
                            in1=psum[:, 0:WW],
                            op0=mybir.AluOpType.subtract,
                            op1=mybir.AluOpType.max,
                        )
                        nc.vector.tensor_reduce(
                            dmin_sb[:, m * NCOL + G:m * NCOL + G + 1],
                            wt[:],
                            axis=mybir.AxisListType.X,
                            op=mybir.AluOpType.min,
                        )
                    else:
                        wt = wb_p.tile([P, FD], bf16, tag="wb")
                        nc.vector.scalar_tensor_tensor(
                            out=wt[:],
                            in0=io_sb[:],
                            scalar=cut_sb[:, m:m + 1],
                            in1=psum[:],
                            op0=mybir.AluOpType.subtract,
                            op1=mybir.AluOpType.max,
                        )
                        nc.vector.tensor_reduce(
                            dmin_sb[:, m * NCOL + G:m * NCOL + G + 1],
                            wt[:],
                            axis=mybir.AxisListType.X,
                            op=mybir.AluOpType.min,
                        )
            nc.gpsimd.dma_start(out=dmin_d[:], in_=dmin_sb[:])
    nc.compile()
    return nc


_BUILDERS = {"l12": _build_l12, "l3": _build_l3}


def _get_program(name):
    if name not in _programs:
        _programs[name] = _BUILDERS[name]()
    return _programs[name]


TIMINGS = []  # (name, exec_time_ns) per launch, appended by _run


def _run(name, in_maps, trace=None):
    from concourse.bass_utils import run_bass_kernel_spmd

    if trace is None:
        trace = bool(int(os.environ.get("KERNEL_TRACE", "0")))
    nc = _get_program(name)
    res = run_bass_kernel_spmd(
        nc, in_maps, core_ids=list(range(NCORES)), trace=trace
    )
    TIMINGS.append((name, res.exec_time_ns))
    return res


def _augmented(data):
    """U (lhs rows) and V (rhs cols) of the K=68 augmented distance GEMM.

    bf16 operands with sq split into a bf16 hi+lo pair: d2 error ~0.04 abs
    (~5e-4 relative at the dc^2 scale), far inside every decision margin.
    """
    import ml_dtypes

    bf = ml_dtypes.bfloat16
    sq = np.einsum("ij,ij->i", data, data, dtype=np.float32).astype(np.float32)
    sqh = sq.astype(bf)
    sql = (sq - sqh.astype(np.float32)).astype(bf)
    ones = np.ones((N, 1), bf)
    zcol = lambda a: a[:, None]
    U = np.concatenate(
        [(-2.0 * data).astype(bf), zcol(sqh), zcol(sql), ones, ones], axis=1
    )
    V = np.concatenate(
        [data.astype(bf), ones, ones, zcol(sqh), zcol(sql)], axis=1
    )
    return U, V, sq


def _erf(x):
    """Abramowitz-Stegun 7.1.26 vectorized erf (|err| < 1.5e-7)."""
    s = np.sign(x)
    x = np.abs(x)
    t = 1.0 / (1.0 + 0.3275911 * x)
    y = 1.0 - (
        ((((1.061405429 * t - 1.453152027) * t) + 1.421413741) * t - 0.284496736)
        * t
        + 0.254829592
    ) * t * np.exp(-x * x)
    return s * y


def _phi(z):
    return 0.5 * (1.0 + _erf(z / np.sqrt(2.0)))


NGRID = 256


def _cv_corrections(sq):
    """Control-variate count corrections for the fixed count sample.

    Model P(d2 < t | sq_i, sq_j) ~ Phi((t - sq_i - sq_j)/(2 sqrt(sq_i sq_j/D)))
    and subtract the predicted row/col selection bias of the sampled
    rows/cols relative to the full point set.  Returned in sigmoid-count
    units; the caller transforms to tanh units.
    """
    sq64 = sq.astype(np.float64)
    step = N // NGRID
    grid = np.sort(sq64)[step // 2::step][:NGRID]

    def h(t, svals):
        s = svals[:, None]
        sp = grid[None, :]
        z = (t - s - sp) / (2.0 * np.sqrt(np.maximum(s * sp, 1e-9) / D))
        return _phi(z).mean(axis=1)

    dvec = np.zeros(NT)
    for b, (m, g) in enumerate(L1_GROUPS):
        t = float(DC2_GRID[b])
        h_all = h(t, grid).mean()
        d_row = h(t, sq64[m * P:(m + 1) * P]).mean() - h_all
        d_col = h(t, sq64[g * FD:g * FD + L1_W]).mean() - h_all
        dvec[b] = (d_row + d_col) * (P * L1_W)
    return dvec.astype(np.float64).reshape(1, NT)


def _host_fallback(data, rho_t, delta_t):
    """Pure-numpy reference path (only used if device assumptions break)."""
    data = np.asarray(data, np.float32)
    sq = np.sum(data * data, axis=1)
    d2 = sq[:, None] + sq[None, :] - 2.0 * (data @ data.T)
    dist = np.sqrt(np.maximum(d2, 0.0), dtype=np.float32)
    dc = np.percentile(dist, PCT)
    rho = np.exp(-((dist / dc) ** 2)).sum(axis=1).astype(np.float32)
    higher = rho[None, :] > rho[:, None]
    masked = np.where(higher, dist, np.inf)
    delta_m = masked.min(axis=1)
    nhd_m = masked.argmin(axis=1)
    has = higher.any(axis=1)
    delta = np.where(has, delta_m, dist.max(axis=1))
    nhd = np.where(has, nhd_m, np.arange(N))
    return _finish_labels(rho, delta, nhd, rho_t, delta_t)


def _finish_labels(rho, delta, nhd, rho_t, delta_t):
    is_center = (rho > rho_t) & (delta > delta_t)
    center_rank = np.cumsum(is_center.astype(np.int32)) - 1
    labels = np.where(is_center, center_rank, -1).astype(np.int32)
    order = np.argsort(-rho, kind="stable")
    for i in order:
        if labels[i] < 0:
            labels[i] = labels[nhd[i]]
    return labels


def kernel(data, rho_threshold, delta_threshold):
    data = np.ascontiguousarray(np.asarray(data, dtype=np.float32))
    assert data.shape == (N, D)
    rho_t = float(np.asarray(rho_threshold))
    delta_t = float(np.asarray(delta_threshold))

    U, V, sq = _augmented(data)
    VT = V.T  # [K, N]

    # ---- L12: counts -> on-device dc^2 -> rho (single launch) ----------
    # tanh step: arg = -0.5*SIG_ALPHA*(d2 - t_b); accum = 2*count - L1_W
    thr = np.broadcast_to(
        (0.5 * SIG_ALPHA * DC2_GRID).astype(np.float32)[None, :], (P, NT)
    ).copy()
    tvec = DC2_GRID.astype(np.float32).reshape(1, NT)
    dvec_sig = _cv_corrections(sq)  # sigmoid-count units, [1, NT] f64
    dvec = (2.0 * dvec_sig - float(P * L1_W)).astype(np.float32)
    uvc = np.ascontiguousarray(np.concatenate([U[0:ROWS].T, VT], axis=1)[:, 0:ROWS])
    in_maps = [
        {
            "uv": np.ascontiguousarray(
                np.concatenate([U[c * ROWS:(c + 1) * ROWS].T, VT], axis=1)
            ),
            "uvc": uvc,
            "thr": thr,
            "tvec": tvec,
            "dvec": dvec,
        }
        for c in range(NCORES)
    ]
    r12 = _run("l12", in_maps)

    # validate the on-device dc interpolation from the counts output
    # (device accums are in tanh units: q = 2*(count - dvec_sig))
    q = (
        r12.results[0]["counts"].astype(np.float64).sum(axis=0)
        - dvec[0].astype(np.float64)
    )
    brackets = [b for b in range(NT - 1) if q[b] <= CSTAR2 < q[b + 1]]
    if len(brackets) != 1 or not np.all(np.diff(q) > 0):
        return _host_fallback(data, rho_t, delta_t)

    rho = np.empty(N, np.float32)
    for c in range(NCORES):
        out = r12.results[c]["rho"]  # [P, RB]
        rho[c * ROWS:(c + 1) * ROWS] = out.T.reshape(-1)
    if not np.all(np.isfinite(rho)) or rho.min() < 0.5 or rho.max() > N + 1:
        return _host_fallback(data, rho_t, delta_t)

    # ---- host: sort by rho desc; prefix cutoffs ------------------------
    order = np.argsort(-rho, kind="stable")
    rho_sorted = rho[order]
    # c_i = #points with rho strictly greater (ties excluded)
    cuts = np.searchsorted(-rho_sorted, -rho_sorted, side="left").astype(np.int64)

    data_p = data[order]
    sq_p = sq[order]
    Up = U[order]
    Vp = V[order]
    rhs_p = np.ascontiguousarray(Vp.T)

    # round-robin block interleave: core c <- sorted blocks 8m + c
    NB = N // P  # 64 sorted row-blocks
    blk_rows = np.arange(N).reshape(NB, P)
    core_rows = [blk_rows[np.arange(RB) * NCORES + c].reshape(-1) for c in range(NCORES)]

    # io[p, col] = MASK_SCALE * (col - WW): window cols [WW, 2*WW) get
    # iota 0..WW-1; prefix cols (odd m) stay deeply negative.
    io_in = np.broadcast_to(
        (MASK_SCALE * (np.arange(FD, dtype=np.float64) - WW)).astype(np.float32)[None, :],
        (P, FD),
    ).copy()
    in_maps = []
    for c in range(NCORES):
        rows = core_rows[c]
        cutsc = np.empty((P, RB), np.float32)
        for m in range(RB):
            base = (m // 2) * FD + WW * (m % 2)
            cutrel = np.clip(cuts[rows[m * P:(m + 1) * P]] - base, 0, WW)
            cutsc[:, m] = MASK_SCALE * (cutrel.astype(np.float64) - 0.5)
        in_maps.append(
            {
                "uv": np.ascontiguousarray(
                    np.concatenate([Up[rows].T, rhs_p], axis=1)
                ),
                "io": io_in,
                "cut": cutsc,
            }
        )
    r3 = _run("l3", in_maps)
    # dmin[i] holds per-source minima; slot k < gb = full group k,
    # slot G = boundary group (odd m: prefix+window merged)
    dmin = np.full((N, NCOL), np.inf, np.float32)
    for c in range(NCORES):
        out = r3.results[c]["dmin"]  # [P, RB*NCOL]
        rows = core_rows[c]
        for m in range(RB):
            gb = m // 2
            blk = rows[m * P:(m + 1) * P]
            for g in range(gb):
                dmin[blk, g] = out[:, m * NCOL + g]
            dmin[blk, G] = out[:, m * NCOL + G]

    # ---- host: delta, fallback rows, centers, nhd (lazy), labels -------
    delta2_sorted = dmin.min(axis=1)

    # rho-tie rows whose cutoff dips below their block's boundary group: the
    # device's full-group reduce included a few extra columns; fix exactly.
    win_base = ((np.arange(N) // P) // NCORES) * WW  # 1024*m per sorted row
    straddle_fix = {}
    for i in np.nonzero(cuts < win_base)[0]:
        cut = int(cuts[i])
        if cut == 0:
            delta2_sorted[i] = np.inf
            continue
        d2row = sq_p[i] + sq_p[:cut] - 2.0 * (data_p[:cut] @ data_p[i])
        j = int(np.argmin(d2row))
        delta2_sorted[i] = d2row[j]
        straddle_fix[i] = j

    empty = delta2_sorted >= EMPTY_SENTINEL  # no higher-density point
    delta_sorted = np.sqrt(np.maximum(delta2_sorted, 0.0), dtype=np.float32)
    for i in np.nonzero(empty)[0]:
        d2row = sq_p[i] + sq_p - 2.0 * (data_p @ data_p[i])
        delta_sorted[i] = np.sqrt(max(float(np.max(np.maximum(d2row, 0.0))), 0.0))

    delta = np.empty(N, np.float32)
    delta[order] = delta_sorted

    is_center = (rho > rho_t) & (delta > delta_t)
    center_rank = np.cumsum(is_center.astype(np.int32)) - 1
    labels = np.where(is_center, center_rank, -1).astype(np.int32)

    need_nhd = ~is_center[order]  # sorted positions whose label must propagate
    nhd = np.arange(N, dtype=np.int64)  # default: self (matches reference)
    for i in np.nonzero(need_nhd)[0]:
        if empty[i]:
            continue  # nhd stays self, as in reference
        if i in straddle_fix:
            nhd[order[i]] = order[straddle_fix[i]]
            continue
        k = int(np.argmin(dmin[i]))
        m = (i // P) // NCORES
        gb = m // 2
        w_lo = WW * (m % 2)
        if k == G:
            c0, clen = gb * FD, w_lo + WW
        else:
            c0, clen = k * FD, FD
        end_local = int(np.clip(cuts[i] - c0, 0, clen))
        cols = slice(c0, c0 + end_local)
        d2part = sq_p[i] + sq_p[cols] - 2.0 * (data_p[cols] @ data_p[i])
        j_local = int(np.argmin(d2part))
        nhd[order[i]] = order[c0 + j_local]

    for i in order:
        if labels[i] < 0:
            labels[i] = labels[nhd[i]]
    return labels.astype(np.int32)


# revision 5
# speedup vs baseline: 1.0375x; 1.0375x over previous
"""CFSDP (density-peaks clustering) on 8 Trainium2 NeuronCores.

Pipeline (N=8192 points, D=64, row-sharded 1024 rows/core):
  d2(i,j) = ||xi-xj||^2 via one K=68 augmented matmul per tile:
      u_i = (-2*x_i, sqh_i, sql_i, 1, 1),  v_j = (x_j, 1, 1, sqh_j, sql_j)
  All O(N^2) math runs on squared distances (sqrt is monotone, so order
  stats / argmin / percentile commute with it):
    L12 launch: hard threshold counts on DVE (tensor_scalar is_le +
        accum) for 4 thresholds around the predicted 2%-quantile ->
        on-device dc^2 interpolation -> rho via ACT Exp + accumulate.
        Inputs are split into per-region tiles (count windows first, on
        the gpsimd SWDGE queue which spreads over all 16 DMA engines) so
        the count matmuls don't wait on the bulk load.
    host: stable-sort rows by rho desc; per-row prefix cutoffs.
    L3 launch: delta^2 = min d2 over the sorted prefix.  The per-row
        window mask is ADDED INTO PSUM by a second accumulating matmul
        (lhsT = BIG*I, rhs = host-built 0/1 mask), so each col-group
        needs exactly one DVE min-reduce - no mask build, no extra adds.
  Host finishes: delta fallback (row max) for top-density rows, nhd argmin
  (lazy, only for non-center points), center ranks, label propagation scan.
"""

import os
import numpy as np

N = 8192
D = 64
NCORES = 8
ROWS = N // NCORES          # 1024 rows per core
P = 128                     # partitions
RB = ROWS // P              # 8 row-blocks per core
FD = 2048                   # free-dim group (4 PSUM banks)
G = N // FD                 # 4 col-groups per row
K = D + 4                   # 68 (augmented contraction dim, sq split hi+lo)
MM_N = 512                  # cols per matmul (one PSUM bank output)
MM_PER_G = FD // MM_N       # 4

NT = 4                      # percentile-count thresholds
L1_W = 1024                 # cols counted per threshold
DC2_CENTER = 86.2           # chi^2_64-predicted 2%-quantile of d2 (randn data)
DC2_GRID = (DC2_CENTER * (1.0 + (np.arange(NT) - (NT - 1) / 2) * 0.023)).astype(
    np.float64
)                           # +-3.5% bracket, 2.3% spacing
PCT = 2.0
WW = 1024                   # L3 boundary mask window width
NCOL = G + 1                # L3 output cols per block (G group slots + window)
MASK_BIG = 1.0e4            # L3 mask penalty (bf16: 9984), >> max d2 ~400
EMPTY_SENTINEL = 5.0e3      # boundary min >= this => empty prefix window

# threshold b is counted on group (m, g) of every core (1/16 of the matrix
# per threshold => ~4.2M samples each; different rows+cols per threshold)
L1_GROUPS = [(b % RB, 1 + b % (G - 1)) for b in range(NT)]  # g>0: diag-free
DC2_STEP = float(DC2_CENTER * 0.023)
M_TOT = float(N) * float(N)
K_POS = PCT / 100.0 * (M_TOT - 1.0)
P_OFF = (K_POS - N) / (M_TOT - N)      # diag-free target CDF
CSTAR = float(P_OFF * P * L1_W)        # target count over the device sample

_programs: dict = {}


def _build_l12():
    """Merged count + rho launch: dc^2 is computed ON DEVICE.

    Every core counts the SAME sample (rows 0..1023 via the shared `uvc`
    lhsT, diag-free col groups), so each core independently derives an
    identical dc^2 - no collectives.  Counts are exact hard thresholds on
    DVE (tensor_scalar is_le with accum_out), keeping ACT free for the
    single Exp table load + rho pass.  The CDF interpolation runs as tiny
    [1,NT] vector ops; a PE ones-matmul does the partition reduction and
    a K=1 fp32 matmul broadcasts -1/dc^2 to all partitions for rho.
    `dvec` carries host-computed control-variate corrections (in counts)
    that cancel the row/col sampling bias of the fixed sample.
    """
    import concourse.mybir as mybir
    import concourse.tile as tile
    from concourse import bacc

    f32 = mybir.dt.float32
    nc = bacc.Bacc("TRN2", debug=False, enable_asserts=False)
    bf16 = mybir.dt.bfloat16
    uvc_d = nc.dram_tensor("uvc", [K, ROWS], bf16, kind="ExternalInput")
    w_d = [
        nc.dram_tensor(f"w{b}", [K, L1_W], bf16, kind="ExternalInput")
        for b in range(NT)
    ]
    ur_d = nc.dram_tensor("ur", [K, ROWS], bf16, kind="ExternalInput")
    vt_d = [
        nc.dram_tensor(f"vt{g}", [K, FD], bf16, kind="ExternalInput")
        for g in range(G)
    ]
    tvec_d = nc.dram_tensor("tvec", [1, NT], f32, kind="ExternalInput")
    dvec_d = nc.dram_tensor("dvec", [1, NT], f32, kind="ExternalInput")
    cnt_d = nc.dram_tensor("counts", [P, NT], f32, kind="ExternalOutput")
    rho_d = nc.dram_tensor("rho", [P, RB], f32, kind="ExternalOutput")

    with tile.TileContext(nc) as tc:
        with (
            tc.tile_pool(name="inp", bufs=1) as inp,
            tc.tile_pool(name="stat", bufs=1) as stat,
            tc.tile_pool(name="trash", bufs=2) as trash_p,
            tc.tile_pool(name="psum", bufs=2, space="PSUM") as psum_p,
        ):
            # count-phase inputs on the SWDGE queue (all 16 DMA engines),
            # bulk rho inputs on the sync HWDGE queue - independent tiles
            # so each matmul waits only for its own region.
            uvc_sb = inp.tile([K, ROWS], bf16, tag="uvc")
            nc.gpsimd.dma_start(out=uvc_sb[:], in_=uvc_d[:])
            w_sb = []
            for b in range(NT):
                t = inp.tile([K, L1_W], bf16, tag=f"w{b}", name=f"w{b}_sb")
                nc.gpsimd.dma_start(out=t[:], in_=w_d[b][:])
                w_sb.append(t)
            tdv_sb = inp.tile([1, 2 * NT], f32, tag="tdv")
            nc.gpsimd.dma_start(out=tdv_sb[:, 0:NT], in_=tvec_d[:])
            nc.gpsimd.dma_start(out=tdv_sb[:, NT:2 * NT], in_=dvec_d[:])
            ur_sb = inp.tile([K, ROWS], bf16, tag="ur")
            nc.sync.dma_start(out=ur_sb[:], in_=ur_d[:])
            vt_sb = []
            for g in range(G):
                t = inp.tile([K, FD], bf16, tag=f"vt{g}", name=f"vt{g}_sb")
                nc.sync.dma_start(out=t[:], in_=vt_d[g][:])
                vt_sb.append(t)
            cnts = stat.tile([P, NT], f32)

            # ---- phase 1: counts over the shared sample (DVE) -----------
            for b, (m, g) in enumerate(L1_GROUPS):
                psum = psum_p.tile([P, FD], f32, tag="psum")
                for j in range(L1_W // MM_N):
                    nc.tensor.matmul(
                        psum[:, j * MM_N:(j + 1) * MM_N],
                        uvc_sb[:, m * P:(m + 1) * P],
                        w_sb[b][:, j * MM_N:(j + 1) * MM_N],
                        start=True,
                        stop=True,
                    )
                t = trash_p.tile([P, L1_W], f32, tag="cntrash")
                nc.vector.tensor_scalar(
                    out=t[:],
                    in0=psum[:, 0:L1_W],
                    scalar1=float(DC2_GRID[b]),
                    scalar2=None,
                    op0=mybir.AluOpType.is_le,
                )
                nc.vector.tensor_reduce(
                    cnts[:, b:b + 1], t[:],
                    axis=mybir.AxisListType.X, op=mybir.AluOpType.add,
                )
            nc.gpsimd.dma_start(out=cnt_d[:], in_=cnts[:])

            # ---- phase 2: dc^2 from counts (identical on every core) ----
            ones_col = stat.tile([P, 1], f32)
            nc.vector.memset(ones_col[:], 1.0)
            ps_tot = psum_p.tile([1, NT], f32, tag="psum")
            nc.tensor.matmul(ps_tot[:], ones_col[:], cnts[:], start=True, stop=True)
            w = stat.tile([1, 8 * NT], f32)  # scratch lanes along free dim
            q = w[:, 0:NT]
            nc.vector.tensor_tensor(
                out=q, in0=ps_tot[:], in1=tdv_sb[:, NT:2 * NT],
                op=mybir.AluOpType.subtract,
            )
            NB_ = NT - 1
            a_ = w[:, NT:NT + NB_]
            nc.vector.tensor_scalar(
                out=a_, in0=q[:, 0:NB_], scalar1=CSTAR, scalar2=None,
                op0=mybir.AluOpType.is_le,
            )
            b_ = w[:, 2 * NT:2 * NT + NB_]
            nc.vector.tensor_scalar(
                out=b_, in0=q[:, 1:NT], scalar1=CSTAR, scalar2=None,
                op0=mybir.AluOpType.is_gt,
            )
            sel = w[:, 3 * NT:3 * NT + NB_]
            nc.vector.tensor_tensor(out=sel, in0=a_, in1=b_, op=mybir.AluOpType.mult)
            den = w[:, 4 * NT:4 * NT + NB_]
            nc.vector.tensor_tensor(
                out=den, in0=q[:, 1:NT], in1=q[:, 0:NB_],
                op=mybir.AluOpType.subtract,
            )
            rec = w[:, 5 * NT:5 * NT + NB_]
            nc.vector.reciprocal(rec, den)
            num = w[:, 6 * NT:6 * NT + NB_]
            nc.vector.tensor_scalar(
                out=num, in0=q[:, 0:NB_], scalar1=-1.0, scalar2=CSTAR,
                op0=mybir.AluOpType.mult, op1=mybir.AluOpType.add,
            )
            fr = w[:, 7 * NT:7 * NT + NB_]
            nc.vector.tensor_tensor(out=fr, in0=num, in1=rec, op=mybir.AluOpType.mult)
            nc.vector.tensor_scalar(
                out=fr, in0=fr, scalar1=float(DC2_STEP), scalar2=None,
                op0=mybir.AluOpType.mult,
            )
            nc.vector.tensor_tensor(
                out=fr, in0=fr, in1=tdv_sb[:, 0:NB_], op=mybir.AluOpType.add
            )
            nc.vector.tensor_tensor(out=fr, in0=fr, in1=sel, op=mybir.AluOpType.mult)
            sc = stat.tile([1, 4], f32)
            nc.vector.tensor_reduce(
                sc[:, 0:1], fr[:], axis=mybir.AxisListType.X, op=mybir.AluOpType.add
            )
            nc.vector.tensor_reduce(
                sc[:, 1:2], sel[:], axis=mybir.AxisListType.X, op=mybir.AluOpType.add
            )
            # guard: if no bracket, fall back to the grid center
            nc.vector.tensor_scalar(
                out=sc[:, 2:3], in0=sc[:, 1:2], scalar1=float(-DC2_CENTER),
                scalar2=float(DC2_CENTER), op0=mybir.AluOpType.mult,
                op1=mybir.AluOpType.add,
            )
            nc.vector.tensor_tensor(
                out=sc[:, 0:1], in0=sc[:, 0:1], in1=sc[:, 2:3],
                op=mybir.AluOpType.add,
            )
            nc.vector.reciprocal(sc[:, 3:4], sc[:, 0:1])
            nc.vector.tensor_scalar(
                out=sc[:, 3:4], in0=sc[:, 3:4], scalar1=-1.0, scalar2=None,
                op0=mybir.AluOpType.mult,
            )
            ones_row = stat.tile([1, P], f32)
            nc.vector.memset(ones_row[:], 1.0)
            ps_b = psum_p.tile([P, 1], f32, tag="psum")
            nc.tensor.matmul(ps_b[:], ones_row[:], sc[:, 3:4], start=True, stop=True)
            scl_sb = stat.tile([P, 1], f32)
            nc.vector.tensor_copy(scl_sb[:], ps_b[:])

            # ---- phase 3: rho ------------------------------------------
            parts = stat.tile([P, RB * G], f32)
            rho_sb = stat.tile([P, RB], f32)
            for m in range(RB):
                for g in range(G):
                    psum = psum_p.tile([P, FD], f32, tag="psum")
                    for j in range(MM_PER_G):
                        nc.tensor.matmul(
                            psum[:, j * MM_N:(j + 1) * MM_N],
                            ur_sb[:, m * P:(m + 1) * P],
                            vt_sb[g][:, j * MM_N:(j + 1) * MM_N],
                            start=True,
                            stop=True,
                        )
                    t = trash_p.tile([P, FD], f32, tag="trash")
                    q2 = m * G + g
                    nc.scalar.activation(
                        t[:],
                        psum[:],
                        mybir.ActivationFunctionType.Exp,
                        bias=0.0,
                        scale=scl_sb[:, 0:1],
                        accum_out=parts[:, q2:q2 + 1],
                    )
                nc.vector.tensor_reduce(
                    rho_sb[:, m:m + 1],
                    parts[:, m * G:(m + 1) * G],
                    axis=mybir.AxisListType.X,
                    op=mybir.AluOpType.add,
                )
            nc.sync.dma_start(out=rho_d[:], in_=rho_sb[:])
    nc.compile()
    return nc


def _build_l3():
    """Delta pass on rho-sorted data (round-robin block interleaving).

    Core c holds sorted row-blocks b = 8m + c (m = 0..7).  For local block
    m: boundary col-group gb = m//2, window base w_lo = 1024*(m%2).
    Structure per block:
      groups g < gb:  plain DVE min-reduce of the [P,2048] psum.
      boundary group (cols [0, w_lo+1024)): the d2 matmuls of the window
        chunks leave the psum banks OPEN (stop=False); a second matmul
        (lhsT = MASK_BIG*I, rhs = per-core 0/1 mask) accumulates the
        penalty, then ONE min-reduce covers prefix+window.
      columns beyond the window are never matmul'd.
    Ties that push a row's cutoff below its block's boundary window are
    patched exactly on the host (straddle_fix), as are empty prefixes
    (boundary min >= EMPTY_SENTINEL).
    """
    import concourse.mybir as mybir
    import concourse.tile as tile
    from concourse import bacc

    f32 = mybir.dt.float32
    nc = bacc.Bacc("TRN2", debug=False, enable_asserts=False)
    bf16 = mybir.dt.bfloat16
    ur_d = nc.dram_tensor("ur", [K, ROWS], bf16, kind="ExternalInput")
    vt_d = [
        nc.dram_tensor(f"vt{g}", [K, FD], bf16, kind="ExternalInput")
        for g in range(G)
    ]
    id_d = nc.dram_tensor("idb", [P, P], bf16, kind="ExternalInput")
    mask_d = nc.dram_tensor("mask", [P, RB * WW], bf16, kind="ExternalInput")
    dmin_d = nc.dram_tensor("dmin", [P, RB * NCOL], f32, kind="ExternalOutput")

    with tile.TileContext(nc) as tc:
        with (
            tc.tile_pool(name="inp", bufs=1) as inp,
            tc.tile_pool(name="stat", bufs=1) as stat,
            tc.tile_pool(name="psum", bufs=2, space="PSUM") as psum_p,
        ):
            ur_sb = inp.tile([K, ROWS], bf16, tag="ur")
            nc.sync.dma_start(out=ur_sb[:], in_=ur_d[:])
            id_sb = inp.tile([P, P], bf16, tag="idb")
            nc.gpsimd.dma_start(out=id_sb[:], in_=id_d[:])
            mask_sb = inp.tile([P, RB * WW], bf16, tag="mask")
            nc.gpsimd.dma_start(
                out=mask_sb[:, 0:2 * WW], in_=mask_d[:, 0:2 * WW]
            )
            vt_sb = []
            for g in range(G):
                t = inp.tile([K, FD], bf16, tag=f"vt{g}", name=f"vt{g}_sb")
                eng = nc.sync if g < 2 else nc.gpsimd
                eng.dma_start(out=t[:], in_=vt_d[g][:])
                vt_sb.append(t)
            nc.gpsimd.dma_start(
                out=mask_sb[:, 2 * WW:RB * WW], in_=mask_d[:, 2 * WW:RB * WW]
            )
            dmin_sb = stat.tile([P, RB * NCOL], f32)

            for m in range(RB):
                gb = m // 2
                w_lo = WW * (m % 2)
                bw = w_lo + WW          # matmul'd cols in the boundary group
                for g in range(gb + 1):
                    ncols = FD if g < gb else bw
                    psum = psum_p.tile([P, FD], f32, tag="psum")
                    for j in range(ncols // MM_N):
                        in_window = g == gb and j * MM_N >= w_lo
                        nc.tensor.matmul(
                            psum[:, j * MM_N:(j + 1) * MM_N],
                            ur_sb[:, m * P:(m + 1) * P],
                            vt_sb[g][:, j * MM_N:(j + 1) * MM_N],
                            start=True,
                            stop=not in_window,
                        )
                        if in_window:
                            wcol = j * MM_N - w_lo
                            nc.tensor.matmul(
                                psum[:, j * MM_N:(j + 1) * MM_N],
                                id_sb[:],
                                mask_sb[:, m * WW + wcol:m * WW + wcol + MM_N],
                                start=False,
                                stop=True,
                            )
                    q = m * NCOL + g
                    if g < gb:
                        nc.vector.tensor_reduce(
                            dmin_sb[:, q:q + 1],
                            psum[:],
                            axis=mybir.AxisListType.X,
                            op=mybir.AluOpType.min,
                        )
                    else:
                        nc.vector.tensor_reduce(
                            dmin_sb[:, m * NCOL + G:m * NCOL + G + 1],
                            psum[:, 0:bw],
                            axis=mybir.AxisListType.X,
                            op=mybir.AluOpType.min,
                        )
            nc.gpsimd.dma_start(out=dmin_d[:], in_=dmin_sb[:])
    nc.compile()
    return nc


_BUILDERS = {"l12": _build_l12, "l3": _build_l3}


def _get_program(name):
    if name not in _programs:
        _programs[name] = _BUILDERS[name]()
    return _programs[name]


TIMINGS = []  # (name, exec_time_ns) per launch, appended by _run


def _run(name, in_maps, trace=None):
    from concourse.bass_utils import run_bass_kernel_spmd

    if trace is None:
        trace = bool(int(os.environ.get("KERNEL_TRACE", "0")))
    nc = _get_program(name)
    res = run_bass_kernel_spmd(
        nc, in_maps, core_ids=list(range(NCORES)), trace=trace
    )
    TIMINGS.append((name, res.exec_time_ns))
    return res


def _augmented(data):
    """U (lhs rows) and V (rhs cols) of the K=68 augmented distance GEMM.

    bf16 operands with sq split into a bf16 hi+lo pair: d2 error ~0.04 abs
    (~5e-4 relative at the dc^2 scale), far inside every decision margin.
    """
    import ml_dtypes

    bf = ml_dtypes.bfloat16
    sq = np.einsum("ij,ij->i", data, data, dtype=np.float32).astype(np.float32)
    sqh = sq.astype(bf)
    sql = (sq - sqh.astype(np.float32)).astype(bf)
    ones = np.ones((N, 1), bf)
    zcol = lambda a: a[:, None]
    U = np.concatenate(
        [(-2.0 * data).astype(bf), zcol(sqh), zcol(sql), ones, ones], axis=1
    )
    V = np.concatenate(
        [data.astype(bf), ones, ones, zcol(sqh), zcol(sql)], axis=1
    )
    return U, V, sq


def _erf(x):
    """Abramowitz-Stegun 7.1.26 vectorized erf (|err| < 1.5e-7)."""
    s = np.sign(x)
    x = np.abs(x)
    t = 1.0 / (1.0 + 0.3275911 * x)
    y = 1.0 - (
        ((((1.061405429 * t - 1.453152027) * t) + 1.421413741) * t - 0.284496736)
        * t
        + 0.254829592
    ) * t * np.exp(-x * x)
    return s * y


def _phi(z):
    return 0.5 * (1.0 + _erf(z / np.sqrt(2.0)))


NGRID = 256


def _cv_corrections(sq):
    """Control-variate count corrections for the fixed count sample.

    Model P(d2 < t | sq_i, sq_j) ~ Phi((t - sq_i - sq_j)/(2 sqrt(sq_i sq_j/D)))
    and subtract the predicted row/col selection bias of the sampled
    rows/cols relative to the full point set.
    """
    sq64 = sq.astype(np.float64)
    step = N // NGRID
    grid = np.sort(sq64)[step // 2::step][:NGRID]

    def h(t, svals):
        s = svals[:, None]
        sp = grid[None, :]
        z = (t - s - sp) / (2.0 * np.sqrt(np.maximum(s * sp, 1e-9) / D))
        return _phi(z).mean(axis=1)

    dvec = np.zeros(NT)
    for b, (m, g) in enumerate(L1_GROUPS):
        t = float(DC2_GRID[b])
        h_all = h(t, grid).mean()
        d_row = h(t, sq64[m * P:(m + 1) * P]).mean() - h_all
        d_col = h(t, sq64[g * FD:g * FD + L1_W]).mean() - h_all
        dvec[b] = (d_row + d_col) * (P * L1_W)
    return dvec.astype(np.float32).reshape(1, NT)


def _host_fallback(data, rho_t, delta_t):
    """Pure-numpy reference path (only used if device assumptions break)."""
    data = np.asarray(data, np.float32)
    sq = np.sum(data * data, axis=1)
    d2 = sq[:, None] + sq[None, :] - 2.0 * (data @ data.T)
    dist = np.sqrt(np.maximum(d2, 0.0), dtype=np.float32)
    dc = np.percentile(dist, PCT)
    rho = np.exp(-((dist / dc) ** 2)).sum(axis=1).astype(np.float32)
    higher = rho[None, :] > rho[:, None]
    masked = np.where(higher, dist, np.inf)
    delta_m = masked.min(axis=1)
    nhd_m = masked.argmin(axis=1)
    has = higher.any(axis=1)
    delta = np.where(has, delta_m, dist.max(axis=1))
    nhd = np.where(has, nhd_m, np.arange(N))
    return _finish_labels(rho, delta, nhd, rho_t, delta_t)


def _finish_labels(rho, delta, nhd, rho_t, delta_t):
    is_center = (rho > rho_t) & (delta > delta_t)
    center_rank = np.cumsum(is_center.astype(np.int32)) - 1
    labels = np.where(is_center, center_rank, -1).astype(np.int32)
    order = np.argsort(-rho, kind="stable")
    for i in order:
        if labels[i] < 0:
            labels[i] = labels[nhd[i]]
    return labels


def kernel(data, rho_threshold, delta_threshold):
    data = np.ascontiguousarray(np.asarray(data, dtype=np.float32))
    assert data.shape == (N, D)
    rho_t = float(np.asarray(rho_threshold))
    delta_t = float(np.asarray(delta_threshold))

    U, V, sq = _augmented(data)
    VT = np.ascontiguousarray(V.T)  # [K, N]

    # ---- L12: counts -> on-device dc^2 -> rho (single launch) ----------
    tvec = DC2_GRID.astype(np.float32).reshape(1, NT)
    dvec = _cv_corrections(sq)
    uvc = np.ascontiguousarray(U[0:ROWS].T)
    w_in = {
        f"w{b}": np.ascontiguousarray(VT[:, g * FD:g * FD + L1_W])
        for b, (m, g) in enumerate(L1_GROUPS)
    }
    vt_in = {
        f"vt{g}": np.ascontiguousarray(VT[:, g * FD:(g + 1) * FD])
        for g in range(G)
    }
    in_maps = [
        {
            "uvc": uvc,
            "ur": np.ascontiguousarray(U[c * ROWS:(c + 1) * ROWS].T),
            "tvec": tvec,
            "dvec": dvec,
            **w_in,
            **vt_in,
        }
        for c in range(NCORES)
    ]
    r12 = _run("l12", in_maps)

    # validate the on-device dc interpolation from the counts output
    q = r12.results[0]["counts"].astype(np.float64).sum(axis=0) - dvec[0].astype(
        np.float64
    )
    brackets = [b for b in range(NT - 1) if q[b] <= CSTAR < q[b + 1]]
    if len(brackets) != 1 or not np.all(np.diff(q) > 0):
        return _host_fallback(data, rho_t, delta_t)

    rho = np.empty(N, np.float32)
    for c in range(NCORES):
        out = r12.results[c]["rho"]  # [P, RB]
        rho[c * ROWS:(c + 1) * ROWS] = out.T.reshape(-1)
    if not np.all(np.isfinite(rho)) or rho.min() < 0.5 or rho.max() > N + 1:
        return _host_fallback(data, rho_t, delta_t)

    # ---- host: sort by rho desc; prefix cutoffs ------------------------
    order = np.argsort(-rho, kind="stable")
    rho_sorted = rho[order]
    # c_i = #points with rho strictly greater (ties excluded)
    cuts = np.searchsorted(-rho_sorted, -rho_sorted, side="left").astype(np.int64)

    data_p = data[order]
    sq_p = sq[order]
    Up = U[order]
    Vp = V[order]
    rhs_p = np.ascontiguousarray(Vp.T)

    # round-robin block interleave: core c <- sorted blocks 8m + c
    NB = N // P  # 64 sorted row-blocks
    blk_rows = np.arange(N).reshape(NB, P)
    core_rows = [blk_rows[np.arange(RB) * NCORES + c].reshape(-1) for c in range(NCORES)]

    import ml_dtypes

    bf = ml_dtypes.bfloat16
    idb = (MASK_BIG * np.eye(P, dtype=np.float32)).astype(bf)
    vt3_in = {
        f"vt{g}": np.ascontiguousarray(rhs_p[:, g * FD:(g + 1) * FD])
        for g in range(G)
    }
    wcol_iota = np.arange(WW)
    in_maps = []
    for c in range(NCORES):
        rows = core_rows[c]
        maskm = np.zeros((P, RB * WW), bf)
        for m in range(RB):
            base = (m // 2) * FD + WW * (m % 2)
            cutrel = np.clip(cuts[rows[m * P:(m + 1) * P]] - base, 0, WW)
            maskm[:, m * WW:(m + 1) * WW] = (
                wcol_iota[None, :] >= cutrel[:, None]
            ).astype(bf)
        in_maps.append(
            {
                "ur": np.ascontiguousarray(Up[rows].T),
                "idb": idb,
                "mask": maskm,
                **vt3_in,
            }
        )
    r3 = _run("l3", in_maps)
    # dmin[i] holds per-source minima; slot k < gb = full group k,
    # slot G = boundary group (prefix + masked window in one reduce)
    dmin = np.full((N, NCOL), np.inf, np.float32)
    for c in range(NCORES):
        out = r3.results[c]["dmin"]  # [P, RB*NCOL]
        rows = core_rows[c]
        for m in range(RB):
            gb = m // 2
            blk = rows[m * P:(m + 1) * P]
            for g in range(gb):
                dmin[blk, g] = out[:, m * NCOL + g]
            dmin[blk, G] = out[:, m * NCOL + G]

    # ---- host: delta, fallback rows, centers, nhd (lazy), labels -------
    delta2_sorted = dmin.min(axis=1)

    # rho-tie rows whose cutoff dips below their block's boundary window:
    # the device's unmasked prefix included a few extra columns; fix exactly.
    win_base = ((np.arange(N) // P) // NCORES) * WW  # 1024*m per sorted row
    straddle_fix = {}
    for i in np.nonzero(cuts < win_base)[0]:
        cut = int(cuts[i])
        if cut == 0:
            delta2_sorted[i] = np.inf
            continue
        d2row = sq_p[i] + sq_p[:cut] - 2.0 * (data_p[:cut] @ data_p[i])
        j = int(np.argmin(d2row))
        delta2_sorted[i] = d2row[j]
        straddle_fix[i] = j

    empty = delta2_sorted >= EMPTY_SENTINEL  # no higher-density point
    delta_sorted = np.sqrt(np.maximum(delta2_sorted, 0.0), dtype=np.float32)
    for i in np.nonzero(empty)[0]:
        d2row = sq_p[i] + sq_p - 2.0 * (data_p @ data_p[i])
        delta_sorted[i] = np.sqrt(max(float(np.max(np.maximum(d2row, 0.0))), 0.0))

    delta = np.empty(N, np.float32)
    delta[order] = delta_sorted

    is_center = (rho > rho_t) & (delta > delta_t)
    center_rank = np.cumsum(is_center.astype(np.int32)) - 1
    labels = np.where(is_center, center_rank, -1).astype(np.int32)

    need_nhd = ~is_center[order]  # sorted positions whose label must propagate
    nhd = np.arange(N, dtype=np.int64)  # default: self (matches reference)
    for i in np.nonzero(need_nhd)[0]:
        if empty[i]:
            continue  # nhd stays self, as in reference
        if i in straddle_fix:
            nhd[order[i]] = order[straddle_fix[i]]
            continue
        k = int(np.argmin(dmin[i]))
        m = (i // P) // NCORES
        gb = m // 2
        w_lo = WW * (m % 2)
        if k == G:
            c0, clen = gb * FD, w_lo + WW
        else:
            c0, clen = k * FD, FD
        end_local = int(np.clip(cuts[i] - c0, 0, clen))
        cols = slice(c0, c0 + end_local)
        d2part = sq_p[i] + sq_p[cols] - 2.0 * (data_p[cols] @ data_p[i])
        j_local = int(np.argmin(d2part))
        nhd[order[i]] = order[c0 + j_local]

    for i in order:
        if labels[i] < 0:
            labels[i] = labels[nhd[i]]
    return labels.astype(np.int32)


# revision 7
# speedup vs baseline: 1.1129x; 1.0727x over previous
"""CFSDP (density-peaks clustering) on 8 Trainium2 NeuronCores.

Pipeline (N=8192 points, D=64, row-sharded 1024 rows/core):
  d2(i,j) = ||xi-xj||^2 via one K=68 augmented matmul per tile:
      u_i = (-2*x_i, sqh_i, sql_i, 1, 1),  v_j = (x_j, 1, 1, sqh_j, sql_j)
  All O(N^2) math runs on squared distances (sqrt is monotone, so order
  stats / argmin / percentile commute with it):
    L12 launch: hard threshold counts on DVE (tensor_scalar is_le +
        accum) for 4 thresholds around the predicted 2%-quantile ->
        on-device dc^2 interpolation -> rho via ACT Exp + accumulate.
        Inputs are split into per-region tiles (count windows first, on
        the gpsimd SWDGE queue which spreads over all 16 DMA engines) so
        the count matmuls don't wait on the bulk load.
    host: stable-sort rows by rho desc; per-row prefix cutoffs.
    L3 launch: delta^2 = min d2 over the sorted prefix.  The per-row
        window mask is ADDED INTO PSUM by a second accumulating matmul
        (lhsT = BIG*I, rhs = host-built 0/1 mask), so each col-group
        needs exactly one DVE min-reduce - no mask build, no extra adds.
  Host finishes: delta fallback (row max) for top-density rows, nhd argmin
  (lazy, only for non-center points), center ranks, label propagation scan.
"""

import os
import numpy as np

N = 8192
D = 64
NCORES = 8
ROWS = N // NCORES          # 1024 rows per core
P = 128                     # partitions
RB = ROWS // P              # 8 row-blocks per core
FD = 2048                   # free-dim group (4 PSUM banks)
G = N // FD                 # 4 col-groups per row
K = D + 4                   # 68 (augmented contraction dim, sq split hi+lo)
MM_N = 512                  # cols per matmul (one PSUM bank output)
MM_PER_G = FD // MM_N       # 4

NT = 4                      # percentile-count thresholds
L1_W = 1024                 # cols counted per threshold
DC2_CENTER = 86.2           # chi^2_64-predicted 2%-quantile of d2 (randn data)
DC2_GRID = (DC2_CENTER * (1.0 + (np.arange(NT) - (NT - 1) / 2) * 0.023)).astype(
    np.float64
)                           # +-3.5% bracket, 2.3% spacing
PCT = 2.0
WW = 1024                   # L3 boundary mask window width
NCOL = G + 1                # L3 output cols per block (G group slots + window)
MASK_BIG = 1.0e4            # L3 mask penalty (bf16: 9984), >> max d2 ~400
EMPTY_SENTINEL = 5.0e3      # boundary min >= this => empty prefix window

# threshold b is counted on group (m, g) of every core (1/16 of the matrix
# per threshold => ~4.2M samples each; different rows+cols per threshold)
L1_GROUPS = [(b % RB, 1 + b % (G - 1)) for b in range(NT)]  # g>0: diag-free
DC2_STEP = float(DC2_CENTER * 0.023)
M_TOT = float(N) * float(N)
K_POS = PCT / 100.0 * (M_TOT - 1.0)
P_OFF = (K_POS - N) / (M_TOT - N)      # diag-free target CDF
CSTAR = float(P_OFF * P * L1_W)        # target count over the device sample

_programs: dict = {}


def _build_l12():
    """Merged count + rho launch: dc^2 is computed ON DEVICE.

    Every core counts the SAME sample (rows 0..1023 via the shared `uvc`
    lhsT, diag-free col groups), so each core independently derives an
    identical dc^2 - no collectives.  Counts are exact hard thresholds on
    DVE (tensor_scalar is_le with accum_out), keeping ACT free for the
    single Exp table load + rho pass.  The CDF interpolation runs as tiny
    [1,NT] vector ops; a PE ones-matmul does the partition reduction and
    a K=1 fp32 matmul broadcasts -1/dc^2 to all partitions for rho.
    `dvec` carries host-computed control-variate corrections (in counts)
    that cancel the row/col sampling bias of the fixed sample.
    """
    import concourse.mybir as mybir
    import concourse.tile as tile
    from concourse import bacc

    f32 = mybir.dt.float32
    nc = bacc.Bacc("TRN2", debug=False, enable_asserts=False)
    bf16 = mybir.dt.bfloat16
    uvc_d = nc.dram_tensor("uvc", [K, ROWS], bf16, kind="ExternalInput")
    w_d = [
        nc.dram_tensor(f"w{b}", [K, L1_W], bf16, kind="ExternalInput")
        for b in range(NT)
    ]
    ur_d = nc.dram_tensor("ur", [K, ROWS], bf16, kind="ExternalInput")
    vt_d = [
        nc.dram_tensor(f"vt{g}", [K, FD], bf16, kind="ExternalInput")
        for g in range(G)
    ]
    tvec_d = nc.dram_tensor("tvec", [1, NT], f32, kind="ExternalInput")
    dvec_d = nc.dram_tensor("dvec", [1, NT], f32, kind="ExternalInput")
    cnt_d = nc.dram_tensor("counts", [P, NT], f32, kind="ExternalOutput")
    rho_d = nc.dram_tensor("rho", [P, RB], f32, kind="ExternalOutput")

    with tile.TileContext(nc) as tc:
        with (
            tc.tile_pool(name="inp", bufs=1) as inp,
            tc.tile_pool(name="stat", bufs=1) as stat,
            tc.tile_pool(name="trash", bufs=2) as trash_p,
            tc.tile_pool(name="psum", bufs=2, space="PSUM") as psum_p,
        ):
            # count-phase inputs on the SWDGE queue (all 16 DMA engines),
            # bulk rho inputs on the sync HWDGE queue - independent tiles
            # so each matmul waits only for its own region.
            uvc_sb = inp.tile([K, ROWS], bf16, tag="uvc")
            nc.sync.dma_start(out=uvc_sb[:], in_=uvc_d[:])
            w_sb = [
                inp.tile([K, L1_W], bf16, tag=f"w{b}", name=f"w{b}_sb")
                for b in range(NT)
            ]
            nc.sync.dma_start(out=w_sb[0][:], in_=w_d[0][:])
            ur_sb = inp.tile([K, ROWS], bf16, tag="ur")
            nc.sync.dma_start(out=ur_sb[:], in_=ur_d[:])
            for b in range(1, NT):
                nc.sync.dma_start(out=w_sb[b][:], in_=w_d[b][:])
            tdv_sb = inp.tile([1, 2 * NT], f32, tag="tdv")
            nc.gpsimd.dma_start(out=tdv_sb[:, 0:NT], in_=tvec_d[:])
            nc.gpsimd.dma_start(out=tdv_sb[:, NT:2 * NT], in_=dvec_d[:])
            vt_sb = []
            for g in range(G):
                t = inp.tile([K, FD], bf16, tag=f"vt{g}", name=f"vt{g}_sb")
                nc.sync.dma_start(out=t[:], in_=vt_d[g][:])
                vt_sb.append(t)
            cnts = stat.tile([P, NT], f32)

            # ---- phase 1: counts over the shared sample (DVE) -----------
            for b, (m, g) in enumerate(L1_GROUPS):
                psum = psum_p.tile([P, FD], f32, tag="psum")
                for j in range(L1_W // MM_N):
                    nc.tensor.matmul(
                        psum[:, j * MM_N:(j + 1) * MM_N],
                        uvc_sb[:, m * P:(m + 1) * P],
                        w_sb[b][:, j * MM_N:(j + 1) * MM_N],
                        start=True,
                        stop=True,
                    )
                t = trash_p.tile([P, L1_W], f32, tag="cntrash")
                nc.vector.tensor_scalar(
                    out=t[:],
                    in0=psum[:, 0:L1_W],
                    scalar1=float(DC2_GRID[b]),
                    scalar2=None,
                    op0=mybir.AluOpType.is_le,
                )
                nc.vector.tensor_reduce(
                    cnts[:, b:b + 1], t[:],
                    axis=mybir.AxisListType.X, op=mybir.AluOpType.add,
                )
            nc.gpsimd.dma_start(out=cnt_d[:], in_=cnts[:])

            # ---- phase 2: dc^2 from counts (identical on every core) ----
            ones_col = stat.tile([P, 1], f32)
            nc.vector.memset(ones_col[:], 1.0)
            ps_tot = psum_p.tile([1, NT], f32, tag="psum")
            nc.tensor.matmul(ps_tot[:], ones_col[:], cnts[:], start=True, stop=True)
            w = stat.tile([1, 8 * NT], f32)  # scratch lanes along free dim
            q = w[:, 0:NT]
            nc.vector.tensor_tensor(
                out=q, in0=ps_tot[:], in1=tdv_sb[:, NT:2 * NT],
                op=mybir.AluOpType.subtract,
            )
            NB_ = NT - 1
            a_ = w[:, NT:NT + NB_]
            nc.vector.tensor_scalar(
                out=a_, in0=q[:, 0:NB_], scalar1=CSTAR, scalar2=None,
                op0=mybir.AluOpType.is_le,
            )
            b_ = w[:, 2 * NT:2 * NT + NB_]
            nc.vector.tensor_scalar(
                out=b_, in0=q[:, 1:NT], scalar1=CSTAR, scalar2=None,
                op0=mybir.AluOpType.is_gt,
            )
            sel = w[:, 3 * NT:3 * NT + NB_]
            nc.vector.tensor_tensor(out=sel, in0=a_, in1=b_, op=mybir.AluOpType.mult)
            den = w[:, 4 * NT:4 * NT + NB_]
            nc.vector.tensor_tensor(
                out=den, in0=q[:, 1:NT], in1=q[:, 0:NB_],
                op=mybir.AluOpType.subtract,
            )
            rec = w[:, 5 * NT:5 * NT + NB_]
            nc.vector.reciprocal(rec, den)
            num = w[:, 6 * NT:6 * NT + NB_]
            nc.vector.tensor_scalar(
                out=num, in0=q[:, 0:NB_], scalar1=-1.0, scalar2=CSTAR,
                op0=mybir.AluOpType.mult, op1=mybir.AluOpType.add,
            )
            fr = w[:, 7 * NT:7 * NT + NB_]
            nc.vector.tensor_tensor(out=fr, in0=num, in1=rec, op=mybir.AluOpType.mult)
            nc.vector.tensor_scalar(
                out=fr, in0=fr, scalar1=float(DC2_STEP), scalar2=None,
                op0=mybir.AluOpType.mult,
            )
            nc.vector.tensor_tensor(
                out=fr, in0=fr, in1=tdv_sb[:, 0:NB_], op=mybir.AluOpType.add
            )
            nc.vector.tensor_tensor(out=fr, in0=fr, in1=sel, op=mybir.AluOpType.mult)
            sc = stat.tile([1, 4], f32)
            nc.vector.tensor_reduce(
                sc[:, 0:1], fr[:], axis=mybir.AxisListType.X, op=mybir.AluOpType.add
            )
            nc.vector.tensor_reduce(
                sc[:, 1:2], sel[:], axis=mybir.AxisListType.X, op=mybir.AluOpType.add
            )
            # guard: if no bracket, fall back to the grid center
            nc.vector.tensor_scalar(
                out=sc[:, 2:3], in0=sc[:, 1:2], scalar1=float(-DC2_CENTER),
                scalar2=float(DC2_CENTER), op0=mybir.AluOpType.mult,
                op1=mybir.AluOpType.add,
            )
            nc.vector.tensor_tensor(
                out=sc[:, 0:1], in0=sc[:, 0:1], in1=sc[:, 2:3],
                op=mybir.AluOpType.add,
            )
            nc.vector.reciprocal(sc[:, 3:4], sc[:, 0:1])
            nc.vector.tensor_scalar(
                out=sc[:, 3:4], in0=sc[:, 3:4], scalar1=-1.0, scalar2=None,
                op0=mybir.AluOpType.mult,
            )
            ones_row = stat.tile([1, P], f32)
            nc.vector.memset(ones_row[:], 1.0)
            ps_b = psum_p.tile([P, 1], f32, tag="psum")
            nc.tensor.matmul(ps_b[:], ones_row[:], sc[:, 3:4], start=True, stop=True)
            scl_sb = stat.tile([P, 1], f32)
            nc.vector.tensor_copy(scl_sb[:], ps_b[:])

            # ---- phase 3: rho ------------------------------------------
            parts = stat.tile([P, RB * G], f32)
            rho_sb = stat.tile([P, RB], f32)
            for m in range(RB):
                for g in range(G):
                    psum = psum_p.tile([P, FD], f32, tag="psum")
                    for j in range(MM_PER_G):
                        nc.tensor.matmul(
                            psum[:, j * MM_N:(j + 1) * MM_N],
                            ur_sb[:, m * P:(m + 1) * P],
                            vt_sb[g][:, j * MM_N:(j + 1) * MM_N],
                            start=True,
                            stop=True,
                        )
                    t = trash_p.tile([P, FD], f32, tag="trash")
                    q2 = m * G + g
                    nc.scalar.activation(
                        t[:],
                        psum[:],
                        mybir.ActivationFunctionType.Exp,
                        bias=0.0,
                        scale=scl_sb[:, 0:1],
                        accum_out=parts[:, q2:q2 + 1],
                    )
                nc.vector.tensor_reduce(
                    rho_sb[:, m:m + 1],
                    parts[:, m * G:(m + 1) * G],
                    axis=mybir.AxisListType.X,
                    op=mybir.AluOpType.add,
                )
            nc.sync.dma_start(out=rho_d[:], in_=rho_sb[:])
    nc.compile()
    return nc


def _build_l3():
    """Delta pass on rho-sorted data (round-robin block interleaving).

    Core c holds sorted row-blocks b = 8m + c (m = 0..7).  For local block
    m: boundary col-group gb = m//2, window base w_lo = 1024*(m%2).
    Structure per block:
      groups g < gb:  plain DVE min-reduce of the [P,2048] psum.
      boundary group (cols [0, w_lo+1024)): the d2 matmuls of the window
        chunks leave the psum banks OPEN (stop=False); a second matmul
        (lhsT = MASK_BIG*I, rhs = per-core 0/1 mask) accumulates the
        penalty, then ONE min-reduce covers prefix+window.
      columns beyond the window are never matmul'd.
    Ties that push a row's cutoff below its block's boundary window are
    patched exactly on the host (straddle_fix), as are empty prefixes
    (boundary min >= EMPTY_SENTINEL).
    """
    import concourse.mybir as mybir
    import concourse.tile as tile
    from concourse import bacc

    f32 = mybir.dt.float32
    nc = bacc.Bacc("TRN2", debug=False, enable_asserts=False)
    bf16 = mybir.dt.bfloat16
    ur_d = nc.dram_tensor("ur", [K, ROWS], bf16, kind="ExternalInput")
    vt_d = [
        nc.dram_tensor(f"vt{g}", [K, FD], bf16, kind="ExternalInput")
        for g in range(G)
    ]
    id_d = nc.dram_tensor("idb", [P, P], bf16, kind="ExternalInput")
    bias_d = nc.dram_tensor("biasc", [P, RB], f32, kind="ExternalInput")
    dmin_d = nc.dram_tensor("dmin", [P, RB * NCOL], f32, kind="ExternalOutput")

    with tile.TileContext(nc) as tc:
        with (
            tc.tile_pool(name="inp", bufs=1) as inp,
            tc.tile_pool(name="stat", bufs=1) as stat,
            tc.tile_pool(name="psum", bufs=2, space="PSUM") as psum_p,
        ):
            ur_sb = inp.tile([K, ROWS], bf16, tag="ur")
            nc.sync.dma_start(out=ur_sb[:], in_=ur_d[:])
            id_sb = inp.tile([P, P], bf16, tag="idb")
            nc.gpsimd.dma_start(out=id_sb[:], in_=id_d[:])
            bias_sb = inp.tile([P, RB], f32, tag="biasc")
            nc.gpsimd.dma_start(out=bias_sb[:], in_=bias_d[:])
            vt_sb = []
            for g in range(G):
                t = inp.tile([K, FD], bf16, tag=f"vt{g}", name=f"vt{g}_sb")
                nc.sync.dma_start(out=t[:], in_=vt_d[g][:])
                vt_sb.append(t)
            dmin_sb = stat.tile([P, RB * NCOL], f32)
            ones_t = stat.tile([P, WW], f32)
            nc.vector.memset(ones_t[:], 1.0)
            iota_t = stat.tile([P, WW], f32)
            nc.vector.tensor_tensor_scan(
                out=iota_t[:],
                data0=ones_t[:],
                data1=ones_t[:],
                initial=-1.0,
                op0=mybir.AluOpType.mult,
                op1=mybir.AluOpType.add,
            )
            mask_sb = stat.tile([P, RB * WW], bf16, tag="mask")
            for m in range(RB):
                nc.scalar.activation(
                    mask_sb[:, m * WW:(m + 1) * WW],
                    iota_t[:],
                    mybir.ActivationFunctionType.Sigmoid,
                    bias=bias_sb[:, m:m + 1],
                    scale=2.0e4,
                )

            for m in range(RB):
                gb = m // 2
                w_lo = WW * (m % 2)
                bw = w_lo + WW          # matmul'd cols in the boundary group
                for g in range(gb + 1):
                    ncols = FD if g < gb else bw
                    psum = psum_p.tile([P, FD], f32, tag="psum")
                    for j in range(ncols // MM_N):
                        in_window = g == gb and j * MM_N >= w_lo
                        nc.tensor.matmul(
                            psum[:, j * MM_N:(j + 1) * MM_N],
                            ur_sb[:, m * P:(m + 1) * P],
                            vt_sb[g][:, j * MM_N:(j + 1) * MM_N],
                            start=True,
                            stop=not in_window,
                        )
                        if in_window:
                            wcol = j * MM_N - w_lo
                            nc.tensor.matmul(
                                psum[:, j * MM_N:(j + 1) * MM_N],
                                id_sb[:],
                                mask_sb[:, m * WW + wcol:m * WW + wcol + MM_N],
                                start=False,
                                stop=True,
                            )
                    q = m * NCOL + g
                    if g < gb:
                        nc.vector.tensor_reduce(
                            dmin_sb[:, q:q + 1],
                            psum[:],
                            axis=mybir.AxisListType.X,
                            op=mybir.AluOpType.min,
                        )
                    else:
                        nc.vector.tensor_reduce(
                            dmin_sb[:, m * NCOL + G:m * NCOL + G + 1],
                            psum[:, 0:bw],
                            axis=mybir.AxisListType.X,
                            op=mybir.AluOpType.min,
                        )
            nc.gpsimd.dma_start(out=dmin_d[:], in_=dmin_sb[:])
    nc.compile()
    return nc


_BUILDERS = {"l12": _build_l12, "l3": _build_l3}


def _get_program(name):
    if name not in _programs:
        _programs[name] = _BUILDERS[name]()
    return _programs[name]


TIMINGS = []  # (name, exec_time_ns) per launch, appended by _run


def _run(name, in_maps, trace=None):
    from concourse.bass_utils import run_bass_kernel_spmd

    if trace is None:
        trace = bool(int(os.environ.get("KERNEL_TRACE", "0")))
    nc = _get_program(name)
    res = run_bass_kernel_spmd(
        nc, in_maps, core_ids=list(range(NCORES)), trace=trace
    )
    TIMINGS.append((name, res.exec_time_ns))
    return res


def _augmented(data):
    """U (lhs rows) and V (rhs cols) of the K=68 augmented distance GEMM.

    bf16 operands with sq split into a bf16 hi+lo pair: d2 error ~0.04 abs
    (~5e-4 relative at the dc^2 scale), far inside every decision margin.
    """
    import ml_dtypes

    bf = ml_dtypes.bfloat16
    sq = np.einsum("ij,ij->i", data, data, dtype=np.float32).astype(np.float32)
    sqh = sq.astype(bf)
    sql = (sq - sqh.astype(np.float32)).astype(bf)
    ones = np.ones((N, 1), bf)
    zcol = lambda a: a[:, None]
    U = np.concatenate(
        [(-2.0 * data).astype(bf), zcol(sqh), zcol(sql), ones, ones], axis=1
    )
    V = np.concatenate(
        [data.astype(bf), ones, ones, zcol(sqh), zcol(sql)], axis=1
    )
    return U, V, sq


def _erf(x):
    """Abramowitz-Stegun 7.1.26 vectorized erf (|err| < 1.5e-7)."""
    s = np.sign(x)
    x = np.abs(x)
    t = 1.0 / (1.0 + 0.3275911 * x)
    y = 1.0 - (
        ((((1.061405429 * t - 1.453152027) * t) + 1.421413741) * t - 0.284496736)
        * t
        + 0.254829592
    ) * t * np.exp(-x * x)
    return s * y


def _phi(z):
    return 0.5 * (1.0 + _erf(z / np.sqrt(2.0)))


NGRID = 256


def _cv_corrections(sq):
    """Control-variate count corrections for the fixed count sample.

    Model P(d2 < t | sq_i, sq_j) ~ Phi((t - sq_i - sq_j)/(2 sqrt(sq_i sq_j/D)))
    and subtract the predicted row/col selection bias of the sampled
    rows/cols relative to the full point set.
    """
    sq64 = sq.astype(np.float64)
    step = N // NGRID
    grid = np.sort(sq64)[step // 2::step][:NGRID]

    def h(t, svals):
        s = svals[:, None]
        sp = grid[None, :]
        z = (t - s - sp) / (2.0 * np.sqrt(np.maximum(s * sp, 1e-9) / D))
        return _phi(z).mean(axis=1)

    dvec = np.zeros(NT)
    for b, (m, g) in enumerate(L1_GROUPS):
        t = float(DC2_GRID[b])
        h_all = h(t, grid).mean()
        d_row = h(t, sq64[m * P:(m + 1) * P]).mean() - h_all
        d_col = h(t, sq64[g * FD:g * FD + L1_W]).mean() - h_all
        dvec[b] = (d_row + d_col) * (P * L1_W)
    return dvec.astype(np.float32).reshape(1, NT)


def _host_fallback(data, rho_t, delta_t):
    """Pure-numpy reference path (only used if device assumptions break)."""
    data = np.asarray(data, np.float32)
    sq = np.sum(data * data, axis=1)
    d2 = sq[:, None] + sq[None, :] - 2.0 * (data @ data.T)
    dist = np.sqrt(np.maximum(d2, 0.0), dtype=np.float32)
    dc = np.percentile(dist, PCT)
    rho = np.exp(-((dist / dc) ** 2)).sum(axis=1).astype(np.float32)
    higher = rho[None, :] > rho[:, None]
    masked = np.where(higher, dist, np.inf)
    delta_m = masked.min(axis=1)
    nhd_m = masked.argmin(axis=1)
    has = higher.any(axis=1)
    delta = np.where(has, delta_m, dist.max(axis=1))
    nhd = np.where(has, nhd_m, np.arange(N))
    return _finish_labels(rho, delta, nhd, rho_t, delta_t)


def _finish_labels(rho, delta, nhd, rho_t, delta_t):
    is_center = (rho > rho_t) & (delta > delta_t)
    center_rank = np.cumsum(is_center.astype(np.int32)) - 1
    labels = np.where(is_center, center_rank, -1).astype(np.int32)
    order = np.argsort(-rho, kind="stable")
    for i in order:
        if labels[i] < 0:
            labels[i] = labels[nhd[i]]
    return labels


def kernel(data, rho_threshold, delta_threshold):
    data = np.ascontiguousarray(np.asarray(data, dtype=np.float32))
    assert data.shape == (N, D)
    rho_t = float(np.asarray(rho_threshold))
    delta_t = float(np.asarray(delta_threshold))

    U, V, sq = _augmented(data)
    VT = np.ascontiguousarray(V.T)  # [K, N]

    # ---- L12: counts -> on-device dc^2 -> rho (single launch) ----------
    tvec = DC2_GRID.astype(np.float32).reshape(1, NT)
    dvec = _cv_corrections(sq)
    uvc = np.ascontiguousarray(U[0:ROWS].T)
    w_in = {
        f"w{b}": np.ascontiguousarray(VT[:, g * FD:g * FD + L1_W])
        for b, (m, g) in enumerate(L1_GROUPS)
    }
    vt_in = {
        f"vt{g}": np.ascontiguousarray(VT[:, g * FD:(g + 1) * FD])
        for g in range(G)
    }
    in_maps = [
        {
            "uvc": uvc,
            "ur": np.ascontiguousarray(U[c * ROWS:(c + 1) * ROWS].T),
            "tvec": tvec,
            "dvec": dvec,
            **w_in,
            **vt_in,
        }
        for c in range(NCORES)
    ]
    r12 = _run("l12", in_maps)

    # validate the on-device dc interpolation from the counts output
    q = r12.results[0]["counts"].astype(np.float64).sum(axis=0) - dvec[0].astype(
        np.float64
    )
    brackets = [b for b in range(NT - 1) if q[b] <= CSTAR < q[b + 1]]
    if len(brackets) != 1 or not np.all(np.diff(q) > 0):
        return _host_fallback(data, rho_t, delta_t)

    rho = np.empty(N, np.float32)
    for c in range(NCORES):
        out = r12.results[c]["rho"]  # [P, RB]
        rho[c * ROWS:(c + 1) * ROWS] = out.T.reshape(-1)
    if not np.all(np.isfinite(rho)) or rho.min() < 0.5 or rho.max() > N + 1:
        return _host_fallback(data, rho_t, delta_t)

    # ---- host: sort by rho desc; prefix cutoffs ------------------------
    order = np.argsort(-rho, kind="stable")
    rho_sorted = rho[order]
    # c_i = #points with rho strictly greater (ties excluded)
    cuts = np.searchsorted(-rho_sorted, -rho_sorted, side="left").astype(np.int64)

    data_p = data[order]
    sq_p = sq[order]
    Up = U[order]
    Vp = V[order]
    rhs_p = np.ascontiguousarray(Vp.T)

    # round-robin block interleave: core c <- sorted blocks 8m + c
    NB = N // P  # 64 sorted row-blocks
    blk_rows = np.arange(N).reshape(NB, P)
    core_rows = [blk_rows[np.arange(RB) * NCORES + c].reshape(-1) for c in range(NCORES)]

    import ml_dtypes

    bf = ml_dtypes.bfloat16
    idb = (MASK_BIG * np.eye(P, dtype=np.float32)).astype(bf)
    vt3_in = {
        f"vt{g}": np.ascontiguousarray(rhs_p[:, g * FD:(g + 1) * FD])
        for g in range(G)
    }
    in_maps = []
    for c in range(NCORES):
        rows = core_rows[c]
        biasc = np.empty((P, RB), np.float32)
        for m in range(RB):
            base = (m // 2) * FD + WW * (m % 2)
            cutrel = np.clip(cuts[rows[m * P:(m + 1) * P]] - base, 0, WW)
            biasc[:, m] = 2.0e4 * (0.5 - cutrel.astype(np.float64))
        in_maps.append(
            {
                "ur": np.ascontiguousarray(Up[rows].T),
                "idb": idb,
                "biasc": biasc,
                **vt3_in,
            }
        )
    r3 = _run("l3", in_maps)
    # dmin[i] holds per-source minima; slot k < gb = full group k,
    # slot G = boundary group (prefix + masked window in one reduce)
    dmin = np.full((N, NCOL), np.inf, np.float32)
    for c in range(NCORES):
        out = r3.results[c]["dmin"]  # [P, RB*NCOL]
        rows = core_rows[c]
        for m in range(RB):
            gb = m // 2
            blk = rows[m * P:(m + 1) * P]
            for g in range(gb):
                dmin[blk, g] = out[:, m * NCOL + g]
            dmin[blk, G] = out[:, m * NCOL + G]

    # ---- host: delta, fallback rows, centers, nhd (lazy), labels -------
    delta2_sorted = dmin.min(axis=1)

    # rho-tie rows whose cutoff dips below their block's boundary window:
    # the device's unmasked prefix included a few extra columns; fix exactly.
    win_base = ((np.arange(N) // P) // NCORES) * WW  # 1024*m per sorted row
    straddle_fix = {}
    for i in np.nonzero(cuts < win_base)[0]:
        cut = int(cuts[i])
        if cut == 0:
            delta2_sorted[i] = np.inf
            continue
        d2row = sq_p[i] + sq_p[:cut] - 2.0 * (data_p[:cut] @ data_p[i])
        j = int(np.argmin(d2row))
        delta2_sorted[i] = d2row[j]
        straddle_fix[i] = j

    empty = delta2_sorted >= EMPTY_SENTINEL  # no higher-density point
    delta_sorted = np.sqrt(np.maximum(delta2_sorted, 0.0), dtype=np.float32)
    for i in np.nonzero(empty)[0]:
        d2row = sq_p[i] + sq_p - 2.0 * (data_p @ data_p[i])
        delta_sorted[i] = np.sqrt(max(float(np.max(np.maximum(d2row, 0.0))), 0.0))

    delta = np.empty(N, np.float32)
    delta[order] = delta_sorted

    is_center = (rho > rho_t) & (delta > delta_t)
    center_rank = np.cumsum(is_center.astype(np.int32)) - 1
    labels = np.where(is_center, center_rank, -1).astype(np.int32)

    need_nhd = ~is_center[order]  # sorted positions whose label must propagate
    nhd = np.arange(N, dtype=np.int64)  # default: self (matches reference)
    for i in np.nonzero(need_nhd)[0]:
        if empty[i]:
            continue  # nhd stays self, as in reference
        if i in straddle_fix:
            nhd[order[i]] = order[straddle_fix[i]]
            continue
        k = int(np.argmin(dmin[i]))
        m = (i // P) // NCORES
        gb = m // 2
        w_lo = WW * (m % 2)
        if k == G:
            c0, clen = gb * FD, w_lo + WW
        else:
            c0, clen = k * FD, FD
        end_local = int(np.clip(cuts[i] - c0, 0, clen))
        cols = slice(c0, c0 + end_local)
        d2part = sq_p[i] + sq_p[cols] - 2.0 * (data_p[cols] @ data_p[i])
        j_local = int(np.argmin(d2part))
        nhd[order[i]] = order[c0 + j_local]

    for i in order:
        if labels[i] < 0:
            labels[i] = labels[nhd[i]]
    return labels.astype(np.int32)


# revision 8
# speedup vs baseline: 1.1609x; 1.0431x over previous
"""CFSDP (density-peaks clustering) on 8 Trainium2 NeuronCores.

Pipeline (N=8192 points, D=64, row-sharded 1024 rows/core):
  d2(i,j) = ||xi-xj||^2 via one K=68 augmented matmul per tile:
      u_i = (-2*x_i, sqh_i, sql_i, 1, 1),  v_j = (x_j, 1, 1, sqh_j, sql_j)
  All O(N^2) math runs on squared distances (sqrt is monotone, so order
  stats / argmin / percentile commute with it):
    L12 launch: hard threshold counts on DVE (tensor_scalar is_le +
        accum) for 4 thresholds around the predicted 2%-quantile ->
        on-device dc^2 interpolation -> rho via ACT Exp + accumulate.
        Inputs are split into per-region tiles (count windows first, on
        the gpsimd SWDGE queue which spreads over all 16 DMA engines) so
        the count matmuls don't wait on the bulk load.
    host: stable-sort rows by rho desc; per-row prefix cutoffs.
    L3 launch: delta^2 = min d2 over the sorted prefix.  The per-row
        window mask is ADDED INTO PSUM by a second accumulating matmul
        (lhsT = BIG*I, rhs = host-built 0/1 mask), so each col-group
        needs exactly one DVE min-reduce - no mask build, no extra adds.
  Host finishes: delta fallback (row max) for top-density rows, nhd argmin
  (lazy, only for non-center points), center ranks, label propagation scan.
"""

import os
import numpy as np

N = 8192
D = 64
NCORES = 8
ROWS = N // NCORES          # 1024 rows per core
P = 128                     # partitions
RB = ROWS // P              # 8 row-blocks per core
FD = 2048                   # free-dim group (4 PSUM banks)
G = N // FD                 # 4 col-groups per row
K = D + 4                   # 68 (augmented contraction dim, sq split hi+lo)
MM_N = 512                  # cols per matmul (one PSUM bank output)
MM_PER_G = FD // MM_N       # 4

NT = 4                      # percentile-count thresholds
L1_W = 512                  # cols counted per threshold
DC2_CENTER = 86.2           # chi^2_64-predicted 2%-quantile of d2 (randn data)
DC2_GRID = (DC2_CENTER * (1.0 + (np.arange(NT) - (NT - 1) / 2) * 0.023)).astype(
    np.float64
)                           # +-3.5% bracket, 2.3% spacing
PCT = 2.0
WW = 1024                   # L3 boundary mask window width
NCOL = G + 1                # L3 output cols per block (G group slots + window)
MASK_BIG = 1.0e4            # L3 mask penalty (bf16: 9984), >> max d2 ~400
EMPTY_SENTINEL = 5.0e3      # boundary min >= this => empty prefix window

# threshold b is counted on group (m, g) of every core (1/16 of the matrix
# per threshold => ~4.2M samples each; different rows+cols per threshold)
L1_GROUPS = [(b % RB, 1 + b % (G - 1)) for b in range(NT)]  # g>0: diag-free
DC2_STEP = float(DC2_CENTER * 0.023)
M_TOT = float(N) * float(N)
K_POS = PCT / 100.0 * (M_TOT - 1.0)
P_OFF = (K_POS - N) / (M_TOT - N)      # diag-free target CDF
CSTAR = float(P_OFF * P * L1_W)        # target count over the device sample
CSTAR2 = 2.0 * CSTAR                   # tanh units: accum = 2*count - L1_W
SIG_ALPHA = 2.0e4                      # sigmoid-equivalent step sharpness

_programs: dict = {}


def _build_l12():
    """Merged count + rho launch: dc^2 is computed ON DEVICE.

    Every core counts the SAME sample (rows 0..1023 via the shared `uvc`
    lhsT, diag-free col groups), so each core independently derives an
    identical dc^2 - no collectives.  Counts are exact hard thresholds on
    DVE (tensor_scalar is_le with accum_out), keeping ACT free for the
    single Exp table load + rho pass.  The CDF interpolation runs as tiny
    [1,NT] vector ops; a PE ones-matmul does the partition reduction and
    a K=1 fp32 matmul broadcasts -1/dc^2 to all partitions for rho.
    `dvec` carries host-computed control-variate corrections (in counts)
    that cancel the row/col sampling bias of the fixed sample.
    """
    import concourse.mybir as mybir
    import concourse.tile as tile
    from concourse import bacc

    f32 = mybir.dt.float32
    nc = bacc.Bacc("TRN2", debug=False, enable_asserts=False)
    bf16 = mybir.dt.bfloat16
    h1_d = nc.dram_tensor("h1", [K, ROWS + L1_W], bf16, kind="ExternalInput")
    h2_d = nc.dram_tensor("h2", [K, (NT - 1) * L1_W], bf16, kind="ExternalInput")
    thr_d = nc.dram_tensor("thr", [P, NT], f32, kind="ExternalInput")
    ur_d = nc.dram_tensor("ur", [K, ROWS], bf16, kind="ExternalInput")
    vt_d = [
        nc.dram_tensor(f"vt{g}", [K, FD], bf16, kind="ExternalInput")
        for g in range(G)
    ]
    tvec_d = nc.dram_tensor("tvec", [1, NT], f32, kind="ExternalInput")
    dvec_d = nc.dram_tensor("dvec", [1, NT], f32, kind="ExternalInput")
    cnt_d = nc.dram_tensor("counts", [P, NT], f32, kind="ExternalOutput")
    rho_d = nc.dram_tensor("rho", [P, RB], f32, kind="ExternalOutput")

    with tile.TileContext(nc) as tc:
        with (
            tc.tile_pool(name="inp", bufs=1) as inp,
            tc.tile_pool(name="stat", bufs=1) as stat,
            tc.tile_pool(name="trash", bufs=2) as trash_p,
            tc.tile_pool(name="psum", bufs=2, space="PSUM") as psum_p,
        ):
            # count-phase inputs on the SWDGE queue (all 16 DMA engines),
            # bulk rho inputs on the sync HWDGE queue - independent tiles
            # so each matmul waits only for its own region.
            h1_sb = inp.tile([K, ROWS + L1_W], bf16, tag="h1")
            nc.sync.dma_start(out=h1_sb[:], in_=h1_d[:])
            h2_sb = inp.tile([K, (NT - 1) * L1_W], bf16, tag="h2")
            nc.sync.dma_start(out=h2_sb[:], in_=h2_d[:])
            uvc_sb = h1_sb[:, 0:ROWS]
            w_ap = [h1_sb[:, ROWS:ROWS + L1_W]] + [
                h2_sb[:, (b - 1) * L1_W:b * L1_W] for b in range(1, NT)
            ]
            thr_sb = inp.tile([P, NT], f32, tag="thr")
            nc.gpsimd.dma_start(out=thr_sb[:], in_=thr_d[:])
            tdv_sb = inp.tile([1, 2 * NT], f32, tag="tdv")
            nc.gpsimd.dma_start(out=tdv_sb[:, 0:NT], in_=tvec_d[:])
            nc.gpsimd.dma_start(out=tdv_sb[:, NT:2 * NT], in_=dvec_d[:])
            ur_sb = inp.tile([K, ROWS], bf16, tag="ur")
            nc.sync.dma_start(out=ur_sb[:], in_=ur_d[:])
            vt_sb = []
            for g in range(G):
                t = inp.tile([K, FD], bf16, tag=f"vt{g}", name=f"vt{g}_sb")
                nc.sync.dma_start(out=t[:], in_=vt_d[g][:])
                vt_sb.append(t)
            cnts = stat.tile([P, NT], f32)

            # ---- phase 1: counts over the shared sample (DVE) -----------
            for b, (m, g) in enumerate(L1_GROUPS):
                psum = psum_p.tile([P, FD], f32, tag="psum")
                for j in range(L1_W // MM_N):
                    nc.tensor.matmul(
                        psum[:, j * MM_N:(j + 1) * MM_N],
                        uvc_sb[:, m * P:(m + 1) * P],
                        w_ap[b][:, j * MM_N:(j + 1) * MM_N],
                        start=True,
                        stop=True,
                    )
                t = trash_p.tile([P, L1_W], f32, tag="cntrash")
                nc.scalar.activation(
                    t[:],
                    psum[:, 0:L1_W],
                    mybir.ActivationFunctionType.Tanh,
                    bias=thr_sb[:, b:b + 1],
                    scale=float(-0.5 * SIG_ALPHA),
                    accum_out=cnts[:, b:b + 1],
                )
            nc.gpsimd.dma_start(out=cnt_d[:], in_=cnts[:])

            # ---- phase 2: dc^2 from counts (identical on every core) ----
            ones_col = stat.tile([P, 1], f32)
            nc.vector.memset(ones_col[:], 1.0)
            ps_tot = psum_p.tile([1, NT], f32, tag="psum")
            nc.tensor.matmul(ps_tot[:], ones_col[:], cnts[:], start=True, stop=True)
            w = stat.tile([1, 8 * NT], f32)  # scratch lanes along free dim
            q = w[:, 0:NT]
            nc.vector.tensor_tensor(
                out=q, in0=ps_tot[:], in1=tdv_sb[:, NT:2 * NT],
                op=mybir.AluOpType.subtract,
            )
            NB_ = NT - 1
            a_ = w[:, NT:NT + NB_]
            nc.vector.tensor_scalar(
                out=a_, in0=q[:, 0:NB_], scalar1=CSTAR2, scalar2=None,
                op0=mybir.AluOpType.is_le,
            )
            b_ = w[:, 2 * NT:2 * NT + NB_]
            nc.vector.tensor_scalar(
                out=b_, in0=q[:, 1:NT], scalar1=CSTAR2, scalar2=None,
                op0=mybir.AluOpType.is_gt,
            )
            sel = w[:, 3 * NT:3 * NT + NB_]
            nc.vector.tensor_tensor(out=sel, in0=a_, in1=b_, op=mybir.AluOpType.mult)
            den = w[:, 4 * NT:4 * NT + NB_]
            nc.vector.tensor_tensor(
                out=den, in0=q[:, 1:NT], in1=q[:, 0:NB_],
                op=mybir.AluOpType.subtract,
            )
            rec = w[:, 5 * NT:5 * NT + NB_]
            nc.vector.reciprocal(rec, den)
            num = w[:, 6 * NT:6 * NT + NB_]
            nc.vector.tensor_scalar(
                out=num, in0=q[:, 0:NB_], scalar1=-1.0, scalar2=CSTAR2,
                op0=mybir.AluOpType.mult, op1=mybir.AluOpType.add,
            )
            fr = w[:, 7 * NT:7 * NT + NB_]
            nc.vector.tensor_tensor(out=fr, in0=num, in1=rec, op=mybir.AluOpType.mult)
            nc.vector.tensor_scalar(
                out=fr, in0=fr, scalar1=float(DC2_STEP), scalar2=None,
                op0=mybir.AluOpType.mult,
            )
            nc.vector.tensor_tensor(
                out=fr, in0=fr, in1=tdv_sb[:, 0:NB_], op=mybir.AluOpType.add
            )
            nc.vector.tensor_tensor(out=fr, in0=fr, in1=sel, op=mybir.AluOpType.mult)
            sc = stat.tile([1, 4], f32)
            nc.vector.tensor_reduce(
                sc[:, 0:1], fr[:], axis=mybir.AxisListType.X, op=mybir.AluOpType.add
            )
            nc.vector.tensor_reduce(
                sc[:, 1:2], sel[:], axis=mybir.AxisListType.X, op=mybir.AluOpType.add
            )
            # guard: if no bracket, fall back to the grid center
            nc.vector.tensor_scalar(
                out=sc[:, 2:3], in0=sc[:, 1:2], scalar1=float(-DC2_CENTER),
                scalar2=float(DC2_CENTER), op0=mybir.AluOpType.mult,
                op1=mybir.AluOpType.add,
            )
            nc.vector.tensor_tensor(
                out=sc[:, 0:1], in0=sc[:, 0:1], in1=sc[:, 2:3],
                op=mybir.AluOpType.add,
            )
            nc.vector.reciprocal(sc[:, 3:4], sc[:, 0:1])
            nc.vector.tensor_scalar(
                out=sc[:, 3:4], in0=sc[:, 3:4], scalar1=-1.0, scalar2=None,
                op0=mybir.AluOpType.mult,
            )
            ones_row = stat.tile([1, P], f32)
            nc.vector.memset(ones_row[:], 1.0)
            ps_b = psum_p.tile([P, 1], f32, tag="psum")
            nc.tensor.matmul(ps_b[:], ones_row[:], sc[:, 3:4], start=True, stop=True)
            scl_sb = stat.tile([P, 1], f32)
            nc.vector.tensor_copy(scl_sb[:], ps_b[:])

            # ---- phase 3: rho ------------------------------------------
            parts = stat.tile([P, RB * G], f32)
            rho_sb = stat.tile([P, RB], f32)
            for m in range(RB):
                for g in range(G):
                    psum = psum_p.tile([P, FD], f32, tag="psum")
                    for j in range(MM_PER_G):
                        nc.tensor.matmul(
                            psum[:, j * MM_N:(j + 1) * MM_N],
                            ur_sb[:, m * P:(m + 1) * P],
                            vt_sb[g][:, j * MM_N:(j + 1) * MM_N],
                            start=True,
                            stop=True,
                        )
                    t = trash_p.tile([P, FD], f32, tag="trash")
                    q2 = m * G + g
                    nc.scalar.activation(
                        t[:],
                        psum[:],
                        mybir.ActivationFunctionType.Exp,
                        bias=0.0,
                        scale=scl_sb[:, 0:1],
                        accum_out=parts[:, q2:q2 + 1],
                    )
                nc.vector.tensor_reduce(
                    rho_sb[:, m:m + 1],
                    parts[:, m * G:(m + 1) * G],
                    axis=mybir.AxisListType.X,
                    op=mybir.AluOpType.add,
                )
            nc.sync.dma_start(out=rho_d[:], in_=rho_sb[:])
    nc.compile()
    return nc


def _build_l3():
    """Delta pass on rho-sorted data (round-robin block interleaving).

    Core c holds sorted row-blocks b = 8m + c (m = 0..7).  For local block
    m: boundary col-group gb = m//2, window base w_lo = 1024*(m%2).
    Structure per block:
      groups g < gb:  plain DVE min-reduce of the [P,2048] psum.
      boundary group (cols [0, w_lo+1024)): the d2 matmuls of the window
        chunks leave the psum banks OPEN (stop=False); a second matmul
        (lhsT = MASK_BIG*I, rhs = per-core 0/1 mask) accumulates the
        penalty, then ONE min-reduce covers prefix+window.
      columns beyond the window are never matmul'd.
    Ties that push a row's cutoff below its block's boundary window are
    patched exactly on the host (straddle_fix), as are empty prefixes
    (boundary min >= EMPTY_SENTINEL).
    """
    import concourse.mybir as mybir
    import concourse.tile as tile
    from concourse import bacc

    f32 = mybir.dt.float32
    nc = bacc.Bacc("TRN2", debug=False, enable_asserts=False)
    bf16 = mybir.dt.bfloat16
    ur_d = nc.dram_tensor("ur", [K, ROWS], bf16, kind="ExternalInput")
    vt_d = [
        nc.dram_tensor(f"vt{g}", [K, FD], bf16, kind="ExternalInput")
        for g in range(G)
    ]
    id_d = nc.dram_tensor("idb", [P, P], bf16, kind="ExternalInput")
    bias_d = nc.dram_tensor("biasc", [P, RB], f32, kind="ExternalInput")
    dmin_d = nc.dram_tensor("dmin", [P, RB * NCOL], f32, kind="ExternalOutput")

    with tile.TileContext(nc) as tc:
        with (
            tc.tile_pool(name="inp", bufs=1) as inp,
            tc.tile_pool(name="stat", bufs=1) as stat,
            tc.tile_pool(name="psum", bufs=2, space="PSUM") as psum_p,
        ):
            ur_sb = inp.tile([K, ROWS], bf16, tag="ur")
            nc.sync.dma_start(out=ur_sb[:], in_=ur_d[:])
            id_sb = inp.tile([P, P], bf16, tag="idb")
            nc.gpsimd.dma_start(out=id_sb[:], in_=id_d[:])
            bias_sb = inp.tile([P, RB], f32, tag="biasc")
            nc.gpsimd.dma_start(out=bias_sb[:], in_=bias_d[:])
            vt_sb = []
            for g in range(G):
                t = inp.tile([K, FD], bf16, tag=f"vt{g}", name=f"vt{g}_sb")
                nc.sync.dma_start(out=t[:], in_=vt_d[g][:])
                vt_sb.append(t)
            dmin_sb = stat.tile([P, RB * NCOL], f32)
            ones_t = stat.tile([P, WW], f32)
            nc.vector.memset(ones_t[:], 1.0)
            iota_t = stat.tile([P, WW], f32)
            nc.vector.tensor_tensor_scan(
                out=iota_t[:],
                data0=ones_t[:],
                data1=ones_t[:],
                initial=-1.0,
                op0=mybir.AluOpType.mult,
                op1=mybir.AluOpType.add,
            )
            mask_sb = stat.tile([P, RB * WW], bf16, tag="mask")
            for m in range(RB):
                nc.scalar.activation(
                    mask_sb[:, m * WW:(m + 1) * WW],
                    iota_t[:],
                    mybir.ActivationFunctionType.Sigmoid,
                    bias=bias_sb[:, m:m + 1],
                    scale=2.0e4,
                )

            for m in range(RB):
                gb = m // 2
                w_lo = WW * (m % 2)
                bw = w_lo + WW          # matmul'd cols in the boundary group
                for g in range(gb + 1):
                    ncols = FD if g < gb else bw
                    psum = psum_p.tile([P, FD], f32, tag="psum")
                    for j in range(ncols // MM_N):
                        in_window = g == gb and j * MM_N >= w_lo
                        nc.tensor.matmul(
                            psum[:, j * MM_N:(j + 1) * MM_N],
                            ur_sb[:, m * P:(m + 1) * P],
                            vt_sb[g][:, j * MM_N:(j + 1) * MM_N],
                            start=True,
                            stop=not in_window,
                        )
                        if in_window:
                            wcol = j * MM_N - w_lo
                            nc.tensor.matmul(
                                psum[:, j * MM_N:(j + 1) * MM_N],
                                id_sb[:],
                                mask_sb[:, m * WW + wcol:m * WW + wcol + MM_N],
                                start=False,
                                stop=True,
                            )
                    q = m * NCOL + g
                    if g < gb:
                        nc.vector.tensor_reduce(
                            dmin_sb[:, q:q + 1],
                            psum[:],
                            axis=mybir.AxisListType.X,
                            op=mybir.AluOpType.min,
                        )
                    else:
                        nc.vector.tensor_reduce(
                            dmin_sb[:, m * NCOL + G:m * NCOL + G + 1],
                            psum[:, 0:bw],
                            axis=mybir.AxisListType.X,
                            op=mybir.AluOpType.min,
                        )
            nc.gpsimd.dma_start(out=dmin_d[:], in_=dmin_sb[:])
    nc.compile()
    return nc


_BUILDERS = {"l12": _build_l12, "l3": _build_l3}


def _get_program(name):
    if name not in _programs:
        _programs[name] = _BUILDERS[name]()
    return _programs[name]


TIMINGS = []  # (name, exec_time_ns) per launch, appended by _run


def _run(name, in_maps, trace=None):
    from concourse.bass_utils import run_bass_kernel_spmd

    if trace is None:
        trace = bool(int(os.environ.get("KERNEL_TRACE", "0")))
    nc = _get_program(name)
    res = run_bass_kernel_spmd(
        nc, in_maps, core_ids=list(range(NCORES)), trace=trace
    )
    TIMINGS.append((name, res.exec_time_ns))
    return res


def _augmented(data):
    """U (lhs rows) and V (rhs cols) of the K=68 augmented distance GEMM.

    bf16 operands with sq split into a bf16 hi+lo pair: d2 error ~0.04 abs
    (~5e-4 relative at the dc^2 scale), far inside every decision margin.
    """
    import ml_dtypes

    bf = ml_dtypes.bfloat16
    sq = np.einsum("ij,ij->i", data, data, dtype=np.float32).astype(np.float32)
    sqh = sq.astype(bf)
    sql = (sq - sqh.astype(np.float32)).astype(bf)
    ones = np.ones((N, 1), bf)
    zcol = lambda a: a[:, None]
    U = np.concatenate(
        [(-2.0 * data).astype(bf), zcol(sqh), zcol(sql), ones, ones], axis=1
    )
    V = np.concatenate(
        [data.astype(bf), ones, ones, zcol(sqh), zcol(sql)], axis=1
    )
    return U, V, sq


def _erf(x):
    """Abramowitz-Stegun 7.1.26 vectorized erf (|err| < 1.5e-7)."""
    s = np.sign(x)
    x = np.abs(x)
    t = 1.0 / (1.0 + 0.3275911 * x)
    y = 1.0 - (
        ((((1.061405429 * t - 1.453152027) * t) + 1.421413741) * t - 0.284496736)
        * t
        + 0.254829592
    ) * t * np.exp(-x * x)
    return s * y


def _phi(z):
    return 0.5 * (1.0 + _erf(z / np.sqrt(2.0)))


NGRID = 256


def _cv_corrections(sq):
    """Control-variate count corrections for the fixed count sample.

    Model P(d2 < t | sq_i, sq_j) ~ Phi((t - sq_i - sq_j)/(2 sqrt(sq_i sq_j/D)))
    and subtract the predicted row/col selection bias of the sampled
    rows/cols relative to the full point set.
    """
    sq64 = sq.astype(np.float64)
    step = N // NGRID
    grid = np.sort(sq64)[step // 2::step][:NGRID]

    def h(t, svals):
        s = svals[:, None]
        sp = grid[None, :]
        z = (t - s - sp) / (2.0 * np.sqrt(np.maximum(s * sp, 1e-9) / D))
        return _phi(z).mean(axis=1)

    dvec = np.zeros(NT)
    for b, (m, g) in enumerate(L1_GROUPS):
        t = float(DC2_GRID[b])
        h_all = h(t, grid).mean()
        d_row = h(t, sq64[m * P:(m + 1) * P]).mean() - h_all
        d_col = h(t, sq64[g * FD:g * FD + L1_W]).mean() - h_all
        dvec[b] = (d_row + d_col) * (P * L1_W)
    return dvec.astype(np.float32).reshape(1, NT)


def _host_fallback(data, rho_t, delta_t):
    """Pure-numpy reference path (only used if device assumptions break)."""
    data = np.asarray(data, np.float32)
    sq = np.sum(data * data, axis=1)
    d2 = sq[:, None] + sq[None, :] - 2.0 * (data @ data.T)
    dist = np.sqrt(np.maximum(d2, 0.0), dtype=np.float32)
    dc = np.percentile(dist, PCT)
    rho = np.exp(-((dist / dc) ** 2)).sum(axis=1).astype(np.float32)
    higher = rho[None, :] > rho[:, None]
    masked = np.where(higher, dist, np.inf)
    delta_m = masked.min(axis=1)
    nhd_m = masked.argmin(axis=1)
    has = higher.any(axis=1)
    delta = np.where(has, delta_m, dist.max(axis=1))
    nhd = np.where(has, nhd_m, np.arange(N))
    return _finish_labels(rho, delta, nhd, rho_t, delta_t)


def _finish_labels(rho, delta, nhd, rho_t, delta_t):
    is_center = (rho > rho_t) & (delta > delta_t)
    center_rank = np.cumsum(is_center.astype(np.int32)) - 1
    labels = np.where(is_center, center_rank, -1).astype(np.int32)
    order = np.argsort(-rho, kind="stable")
    for i in order:
        if labels[i] < 0:
            labels[i] = labels[nhd[i]]
    return labels


def kernel(data, rho_threshold, delta_threshold):
    data = np.ascontiguousarray(np.asarray(data, dtype=np.float32))
    assert data.shape == (N, D)
    rho_t = float(np.asarray(rho_threshold))
    delta_t = float(np.asarray(delta_threshold))

    U, V, sq = _augmented(data)
    VT = np.ascontiguousarray(V.T)  # [K, N]

    # ---- L12: counts -> on-device dc^2 -> rho (single launch) ----------
    tvec = DC2_GRID.astype(np.float32).reshape(1, NT)
    dvec_sig = _cv_corrections(sq).astype(np.float64)
    dvec = (2.0 * dvec_sig - float(P * L1_W)).astype(np.float32)
    thr = np.broadcast_to(
        (0.5 * SIG_ALPHA * DC2_GRID).astype(np.float32)[None, :], (P, NT)
    ).copy()
    uvc = np.ascontiguousarray(U[0:ROWS].T)
    ws = [VT[:, g * FD:g * FD + L1_W] for b, (m, g) in enumerate(L1_GROUPS)]
    h1 = np.ascontiguousarray(np.concatenate([uvc, ws[0]], axis=1))
    h2 = np.ascontiguousarray(np.concatenate(ws[1:], axis=1))
    vt_in = {
        f"vt{g}": np.ascontiguousarray(VT[:, g * FD:(g + 1) * FD])
        for g in range(G)
    }
    in_maps = [
        {
            "h1": h1,
            "h2": h2,
            "thr": thr,
            "ur": np.ascontiguousarray(U[c * ROWS:(c + 1) * ROWS].T),
            "tvec": tvec,
            "dvec": dvec,
            **vt_in,
        }
        for c in range(NCORES)
    ]
    r12 = _run("l12", in_maps)

    # validate the on-device dc interpolation (tanh units: q = 2*(c - dvec_sig))
    q = r12.results[0]["counts"].astype(np.float64).sum(axis=0) - dvec[0].astype(
        np.float64
    )
    brackets = [b for b in range(NT - 1) if q[b] <= CSTAR2 < q[b + 1]]
    if len(brackets) != 1 or not np.all(np.diff(q) > 0):
        return _host_fallback(data, rho_t, delta_t)

    rho = np.empty(N, np.float32)
    for c in range(NCORES):
        out = r12.results[c]["rho"]  # [P, RB]
        rho[c * ROWS:(c + 1) * ROWS] = out.T.reshape(-1)
    if not np.all(np.isfinite(rho)) or rho.min() < 0.5 or rho.max() > N + 1:
        return _host_fallback(data, rho_t, delta_t)

    # ---- host: sort by rho desc; prefix cutoffs ------------------------
    order = np.argsort(-rho, kind="stable")
    rho_sorted = rho[order]
    # c_i = #points with rho strictly greater (ties excluded)
    cuts = np.searchsorted(-rho_sorted, -rho_sorted, side="left").astype(np.int64)

    data_p = data[order]
    sq_p = sq[order]
    Up = U[order]
    Vp = V[order]
    rhs_p = np.ascontiguousarray(Vp.T)

    # round-robin block interleave: core c <- sorted blocks 8m + c
    NB = N // P  # 64 sorted row-blocks
    blk_rows = np.arange(N).reshape(NB, P)
    core_rows = [blk_rows[np.arange(RB) * NCORES + c].reshape(-1) for c in range(NCORES)]

    import ml_dtypes

    bf = ml_dtypes.bfloat16
    idb = (MASK_BIG * np.eye(P, dtype=np.float32)).astype(bf)
    vt3_in = {
        f"vt{g}": np.ascontiguousarray(rhs_p[:, g * FD:(g + 1) * FD])
        for g in range(G)
    }
    in_maps = []
    for c in range(NCORES):
        rows = core_rows[c]
        biasc = np.empty((P, RB), np.float32)
        for m in range(RB):
            base = (m // 2) * FD + WW * (m % 2)
            cutrel = np.clip(cuts[rows[m * P:(m + 1) * P]] - base, 0, WW)
            biasc[:, m] = 2.0e4 * (0.5 - cutrel.astype(np.float64))
        in_maps.append(
            {
                "ur": np.ascontiguousarray(Up[rows].T),
                "idb": idb,
                "biasc": biasc,
                **vt3_in,
            }
        )
    r3 = _run("l3", in_maps)
    # dmin[i] holds per-source minima; slot k < gb = full group k,
    # slot G = boundary group (prefix + masked window in one reduce)
    dmin = np.full((N, NCOL), np.inf, np.float32)
    for c in range(NCORES):
        out = r3.results[c]["dmin"]  # [P, RB*NCOL]
        rows = core_rows[c]
        for m in range(RB):
            gb = m // 2
            blk = rows[m * P:(m + 1) * P]
            for g in range(gb):
                dmin[blk, g] = out[:, m * NCOL + g]
            dmin[blk, G] = out[:, m * NCOL + G]

    # ---- host: delta, fallback rows, centers, nhd (lazy), labels -------
    delta2_sorted = dmin.min(axis=1)

    # rho-tie rows whose cutoff dips below their block's boundary window:
    # the device's unmasked prefix included a few extra columns; fix exactly.
    win_base = ((np.arange(N) // P) // NCORES) * WW  # 1024*m per sorted row
    straddle_fix = {}
    for i in np.nonzero(cuts < win_base)[0]:
        cut = int(cuts[i])
        if cut == 0:
            delta2_sorted[i] = np.inf
            continue
        d2row = sq_p[i] + sq_p[:cut] - 2.0 * (data_p[:cut] @ data_p[i])
        j = int(np.argmin(d2row))
        delta2_sorted[i] = d2row[j]
        straddle_fix[i] = j

    empty = delta2_sorted >= EMPTY_SENTINEL  # no higher-density point
    delta_sorted = np.sqrt(np.maximum(delta2_sorted, 0.0), dtype=np.float32)
    for i in np.nonzero(empty)[0]:
        d2row = sq_p[i] + sq_p - 2.0 * (data_p @ data_p[i])
        delta_sorted[i] = np.sqrt(max(float(np.max(np.maximum(d2row, 0.0))), 0.0))

    delta = np.empty(N, np.float32)
    delta[order] = delta_sorted

    is_center = (rho > rho_t) & (delta > delta_t)
    center_rank = np.cumsum(is_center.astype(np.int32)) - 1
    labels = np.where(is_center, center_rank, -1).astype(np.int32)

    need_nhd = ~is_center[order]  # sorted positions whose label must propagate
    nhd = np.arange(N, dtype=np.int64)  # default: self (matches reference)
    for i in np.nonzero(need_nhd)[0]:
        if empty[i]:
            continue  # nhd stays self, as in reference
        if i in straddle_fix:
            nhd[order[i]] = order[straddle_fix[i]]
            continue
        k = int(np.argmin(dmin[i]))
        m = (i // P) // NCORES
        gb = m // 2
        w_lo = WW * (m % 2)
        if k == G:
            c0, clen = gb * FD, w_lo + WW
        else:
            c0, clen = k * FD, FD
        end_local = int(np.clip(cuts[i] - c0, 0, clen))
        cols = slice(c0, c0 + end_local)
        d2part = sq_p[i] + sq_p[cols] - 2.0 * (data_p[cols] @ data_p[i])
        j_local = int(np.argmin(d2part))
        nhd[order[i]] = order[c0 + j_local]

    for i in order:
        if labels[i] < 0:
            labels[i] = labels[nhd[i]]
    return labels.astype(np.int32)


# revision 10
# speedup vs baseline: 1.1895x; 1.0247x over previous
"""CFSDP (density-peaks clustering) on 8 Trainium2 NeuronCores.

Pipeline (N=8192 points, D=64, row-sharded 1024 rows/core):
  d2(i,j) = ||xi-xj||^2 via one K=68 augmented matmul per tile:
      u_i = (-2*x_i, sqh_i, sql_i, 1, 1),  v_j = (x_j, 1, 1, sqh_j, sql_j)
  All O(N^2) math runs on squared distances (sqrt is monotone, so order
  stats / argmin / percentile commute with it):
    L12 launch: hard threshold counts on DVE (tensor_scalar is_le +
        accum) for 4 thresholds around the predicted 2%-quantile ->
        on-device dc^2 interpolation -> rho via ACT Exp + accumulate.
        Inputs are split into per-region tiles (count windows first, on
        the gpsimd SWDGE queue which spreads over all 16 DMA engines) so
        the count matmuls don't wait on the bulk load.
    host: stable-sort rows by rho desc; per-row prefix cutoffs.
    L3 launch: delta^2 = min d2 over the sorted prefix.  The per-row
        window mask is ADDED INTO PSUM by a second accumulating matmul
        (lhsT = BIG*I, rhs = host-built 0/1 mask), so each col-group
        needs exactly one DVE min-reduce - no mask build, no extra adds.
  Host finishes: delta fallback (row max) for top-density rows, nhd argmin
  (lazy, only for non-center points), center ranks, label propagation scan.
"""

import os
import numpy as np

N = 8192
D = 64
NCORES = 8
ROWS = N // NCORES          # 1024 rows per core
P = 128                     # partitions
RB = ROWS // P              # 8 row-blocks per core
FD = 2048                   # free-dim group (4 PSUM banks)
G = N // FD                 # 4 col-groups per row
K = D + 4                   # 68 (augmented contraction dim, sq split hi+lo)
MM_N = 512                  # cols per matmul (one PSUM bank output)
MM_PER_G = FD // MM_N       # 4

NT = 4                      # percentile-count thresholds
L1_W = 512                  # cols counted per threshold
DC2_CENTER = 86.2           # chi^2_64-predicted 2%-quantile of d2 (randn data)
DC2_GRID = (DC2_CENTER * (1.0 + (np.arange(NT) - (NT - 1) / 2) * 0.023)).astype(
    np.float64
)                           # +-3.5% bracket, 2.3% spacing
PCT = 2.0
WW = 1024                   # L3 boundary mask window width
NCOL = G + 1                # L3 output cols per block (G group slots + window)
MASK_BIG = 1.0e4            # L3 mask penalty (bf16: 9984), >> max d2 ~400
EMPTY_SENTINEL = 5.0e3      # boundary min >= this => empty prefix window

# threshold b is counted on group (m, g) of every core (1/16 of the matrix
# per threshold => ~4.2M samples each; different rows+cols per threshold)
L1_GROUPS = [(b % RB, 1 + b % (G - 1)) for b in range(NT)]  # g>0: diag-free
DC2_STEP = float(DC2_CENTER * 0.023)
M_TOT = float(N) * float(N)
K_POS = PCT / 100.0 * (M_TOT - 1.0)
P_OFF = (K_POS - N) / (M_TOT - N)      # diag-free target CDF
CSTAR = float(P_OFF * P * L1_W)        # target count over the device sample
CSTAR2 = 2.0 * CSTAR                   # tanh units: accum = 2*count - L1_W
SIG_ALPHA = 2.0e4                      # sigmoid-equivalent step sharpness

_programs: dict = {}


def _build_l12():
    """Merged count + rho launch: dc^2 is computed ON DEVICE.

    Every core counts the SAME sample (rows 0..1023 via the shared `uvc`
    lhsT, diag-free col groups), so each core independently derives an
    identical dc^2 - no collectives.  Counts are exact hard thresholds on
    DVE (tensor_scalar is_le with accum_out), keeping ACT free for the
    single Exp table load + rho pass.  The CDF interpolation runs as tiny
    [1,NT] vector ops; a PE ones-matmul does the partition reduction and
    a K=1 fp32 matmul broadcasts -1/dc^2 to all partitions for rho.
    `dvec` carries host-computed control-variate corrections (in counts)
    that cancel the row/col sampling bias of the fixed sample.
    """
    import concourse.mybir as mybir
    import concourse.tile as tile
    from concourse import bacc

    f32 = mybir.dt.float32
    nc = bacc.Bacc("TRN2", debug=False, enable_asserts=False)
    fp8 = mybir.dt.float8e4
    h1_d = nc.dram_tensor("h1", [K, ROWS + L1_W], fp8, kind="ExternalInput")
    h2_d = nc.dram_tensor("h2", [K, (NT - 1) * L1_W], fp8, kind="ExternalInput")
    thr_d = nc.dram_tensor("thr", [P, NT], f32, kind="ExternalInput")
    ur_d = nc.dram_tensor("ur", [K, ROWS], fp8, kind="ExternalInput")
    vt_d = [
        nc.dram_tensor(f"vt{g}", [K, FD], fp8, kind="ExternalInput")
        for g in range(G)
    ]
    tvec_d = nc.dram_tensor("tvec", [1, NT], f32, kind="ExternalInput")
    dvec_d = nc.dram_tensor("dvec", [1, NT], f32, kind="ExternalInput")
    cnt_d = nc.dram_tensor("counts", [P, NT], f32, kind="ExternalOutput")
    rho_d = nc.dram_tensor("rho", [P, RB], f32, kind="ExternalOutput")

    with tile.TileContext(nc) as tc:
        with (
            tc.tile_pool(name="inp", bufs=1) as inp,
            tc.tile_pool(name="stat", bufs=1) as stat,
            tc.tile_pool(name="trash", bufs=2) as trash_p,
            tc.tile_pool(name="psum", bufs=2, space="PSUM") as psum_p,
        ):
            # count-phase inputs on the SWDGE queue (all 16 DMA engines),
            # bulk rho inputs on the sync HWDGE queue - independent tiles
            # so each matmul waits only for its own region.
            h1_sb = inp.tile([K, ROWS + L1_W], fp8, tag="h1")
            nc.sync.dma_start(out=h1_sb[:], in_=h1_d[:])
            h2_sb = inp.tile([K, (NT - 1) * L1_W], fp8, tag="h2")
            nc.sync.dma_start(out=h2_sb[:], in_=h2_d[:])
            uvc_sb = h1_sb[:, 0:ROWS]
            w_ap = [h1_sb[:, ROWS:ROWS + L1_W]] + [
                h2_sb[:, (b - 1) * L1_W:b * L1_W] for b in range(1, NT)
            ]
            thr_sb = inp.tile([P, NT], f32, tag="thr")
            nc.gpsimd.dma_start(out=thr_sb[:], in_=thr_d[:])
            tdv_sb = inp.tile([1, 2 * NT], f32, tag="tdv")
            nc.gpsimd.dma_start(out=tdv_sb[:, 0:NT], in_=tvec_d[:])
            nc.gpsimd.dma_start(out=tdv_sb[:, NT:2 * NT], in_=dvec_d[:])
            ur_sb = inp.tile([K, ROWS], fp8, tag="ur")
            nc.sync.dma_start(out=ur_sb[:], in_=ur_d[:])
            vt_sb = []
            for g in range(G):
                t = inp.tile([K, FD], fp8, tag=f"vt{g}", name=f"vt{g}_sb")
                nc.sync.dma_start(out=t[:], in_=vt_d[g][:])
                vt_sb.append(t)
            cnts = stat.tile([P, NT], f32)

            # ---- phase 1: counts over the shared sample (DVE) -----------
            for b, (m, g) in enumerate(L1_GROUPS):
                psum = psum_p.tile([P, FD], f32, tag="psum")
                for j in range(L1_W // MM_N):
                    nc.tensor.matmul(
                        psum[:, j * MM_N:(j + 1) * MM_N],
                        uvc_sb[:, m * P:(m + 1) * P],
                        w_ap[b][:, j * MM_N:(j + 1) * MM_N],
                        start=True,
                        stop=True,
                    )
                t = trash_p.tile([P, L1_W], f32, tag="cntrash")
                nc.scalar.activation(
                    t[:],
                    psum[:, 0:L1_W],
                    mybir.ActivationFunctionType.Tanh,
                    bias=thr_sb[:, b:b + 1],
                    scale=float(-0.5 * SIG_ALPHA),
                    accum_out=cnts[:, b:b + 1],
                )
            nc.sync.dma_start(out=cnt_d[:], in_=cnts[:])

            # ---- phase 2: dc^2 from counts (identical on every core) ----
            ones_col = stat.tile([P, 1], f32)
            nc.vector.memset(ones_col[:], 1.0)
            ps_tot = psum_p.tile([1, NT], f32, tag="psum")
            nc.tensor.matmul(ps_tot[:], ones_col[:], cnts[:], start=True, stop=True)
            w = stat.tile([1, 8 * NT], f32)  # scratch lanes along free dim
            q = w[:, 0:NT]
            nc.vector.tensor_tensor(
                out=q, in0=ps_tot[:], in1=tdv_sb[:, NT:2 * NT],
                op=mybir.AluOpType.subtract,
            )
            NB_ = NT - 1
            a_ = w[:, NT:NT + NB_]
            nc.vector.tensor_scalar(
                out=a_, in0=q[:, 0:NB_], scalar1=CSTAR2, scalar2=None,
                op0=mybir.AluOpType.is_le,
            )
            b_ = w[:, 2 * NT:2 * NT + NB_]
            nc.vector.tensor_scalar(
                out=b_, in0=q[:, 1:NT], scalar1=CSTAR2, scalar2=None,
                op0=mybir.AluOpType.is_gt,
            )
            sel = w[:, 3 * NT:3 * NT + NB_]
            nc.vector.tensor_tensor(out=sel, in0=a_, in1=b_, op=mybir.AluOpType.mult)
            den = w[:, 4 * NT:4 * NT + NB_]
            nc.vector.tensor_tensor(
                out=den, in0=q[:, 1:NT], in1=q[:, 0:NB_],
                op=mybir.AluOpType.subtract,
            )
            rec = w[:, 5 * NT:5 * NT + NB_]
            nc.vector.reciprocal(rec, den)
            num = w[:, 6 * NT:6 * NT + NB_]
            nc.vector.tensor_scalar(
                out=num, in0=q[:, 0:NB_], scalar1=-1.0, scalar2=CSTAR2,
                op0=mybir.AluOpType.mult, op1=mybir.AluOpType.add,
            )
            fr = w[:, 7 * NT:7 * NT + NB_]
            nc.vector.tensor_tensor(out=fr, in0=num, in1=rec, op=mybir.AluOpType.mult)
            nc.vector.tensor_scalar(
                out=fr, in0=fr, scalar1=float(DC2_STEP), scalar2=None,
                op0=mybir.AluOpType.mult,
            )
            nc.vector.tensor_tensor(
                out=fr, in0=fr, in1=tdv_sb[:, 0:NB_], op=mybir.AluOpType.add
            )
            nc.vector.tensor_tensor(out=fr, in0=fr, in1=sel, op=mybir.AluOpType.mult)
            sc = stat.tile([1, 4], f32)
            nc.vector.tensor_reduce(
                sc[:, 0:1], fr[:], axis=mybir.AxisListType.X, op=mybir.AluOpType.add
            )
            nc.vector.tensor_reduce(
                sc[:, 1:2], sel[:], axis=mybir.AxisListType.X, op=mybir.AluOpType.add
            )
            # guard: if no bracket, fall back to the grid center
            nc.vector.tensor_scalar(
                out=sc[:, 2:3], in0=sc[:, 1:2], scalar1=float(-DC2_CENTER),
                scalar2=float(DC2_CENTER), op0=mybir.AluOpType.mult,
                op1=mybir.AluOpType.add,
            )
            nc.vector.tensor_tensor(
                out=sc[:, 0:1], in0=sc[:, 0:1], in1=sc[:, 2:3],
                op=mybir.AluOpType.add,
            )
            nc.vector.reciprocal(sc[:, 3:4], sc[:, 0:1])
            nc.vector.tensor_scalar(
                out=sc[:, 3:4], in0=sc[:, 3:4], scalar1=-1.0, scalar2=None,
                op0=mybir.AluOpType.mult,
            )
            ones_row = stat.tile([1, P], f32)
            nc.vector.memset(ones_row[:], 1.0)
            ps_b = psum_p.tile([P, 1], f32, tag="psum")
            nc.tensor.matmul(ps_b[:], ones_row[:], sc[:, 3:4], start=True, stop=True)
            scl_sb = stat.tile([P, 1], f32)
            nc.vector.tensor_copy(scl_sb[:], ps_b[:])

            # ---- phase 3: rho ------------------------------------------
            parts = stat.tile([P, RB * G], f32)
            rho_sb = stat.tile([P, RB], f32)
            for m in range(RB):
                for g in range(G):
                    psum = psum_p.tile([P, FD], f32, tag="psum")
                    for j in range(MM_PER_G):
                        nc.tensor.matmul(
                            psum[:, j * MM_N:(j + 1) * MM_N],
                            ur_sb[:, m * P:(m + 1) * P],
                            vt_sb[g][:, j * MM_N:(j + 1) * MM_N],
                            start=True,
                            stop=True,
                        )
                    t = trash_p.tile([P, FD], f32, tag="trash")
                    q2 = m * G + g
                    nc.scalar.activation(
                        t[:],
                        psum[:],
                        mybir.ActivationFunctionType.Exp,
                        bias=0.0,
                        scale=scl_sb[:, 0:1],
                        accum_out=parts[:, q2:q2 + 1],
                    )
                nc.vector.tensor_reduce(
                    rho_sb[:, m:m + 1],
                    parts[:, m * G:(m + 1) * G],
                    axis=mybir.AxisListType.X,
                    op=mybir.AluOpType.add,
                )
            nc.sync.dma_start(out=rho_d[:], in_=rho_sb[:])
    nc.compile()
    return nc


def _build_l3():
    """Delta pass on rho-sorted data (round-robin block interleaving).

    Core c holds sorted row-blocks b = 8m + c (m = 0..7).  For local block
    m: boundary col-group gb = m//2, window base w_lo = 1024*(m%2).
    Structure per block:
      groups g < gb:  plain DVE min-reduce of the [P,2048] psum.
      boundary group (cols [0, w_lo+1024)): the d2 matmuls of the window
        chunks leave the psum banks OPEN (stop=False); a second matmul
        (lhsT = MASK_BIG*I, rhs = per-core 0/1 mask) accumulates the
        penalty, then ONE min-reduce covers prefix+window.
      columns beyond the window are never matmul'd.
    Ties that push a row's cutoff below its block's boundary window are
    patched exactly on the host (straddle_fix), as are empty prefixes
    (boundary min >= EMPTY_SENTINEL).
    """
    import concourse.mybir as mybir
    import concourse.tile as tile
    from concourse import bacc

    f32 = mybir.dt.float32
    nc = bacc.Bacc("TRN2", debug=False, enable_asserts=False)
    bf16 = mybir.dt.bfloat16
    fp8 = mybir.dt.float8e4
    ur_d = nc.dram_tensor("ur", [K, ROWS], fp8, kind="ExternalInput")
    vt_d = [
        nc.dram_tensor(f"vt{g}", [K, FD], fp8, kind="ExternalInput")
        for g in range(G)
    ]
    id_d = nc.dram_tensor("idb", [P, P], bf16, kind="ExternalInput")
    bias_d = nc.dram_tensor("biasc", [P, RB], f32, kind="ExternalInput")
    dmin_d = nc.dram_tensor("dmin", [P, RB * NCOL], f32, kind="ExternalOutput")

    with tile.TileContext(nc) as tc:
        with (
            tc.tile_pool(name="inp", bufs=1) as inp,
            tc.tile_pool(name="stat", bufs=1) as stat,
            tc.tile_pool(name="psum", bufs=2, space="PSUM") as psum_p,
        ):
            ur_sb = inp.tile([K, ROWS], fp8, tag="ur")
            nc.sync.dma_start(out=ur_sb[:], in_=ur_d[:])
            id_sb = inp.tile([P, P], bf16, tag="idb")
            nc.gpsimd.dma_start(out=id_sb[:], in_=id_d[:])
            bias_sb = inp.tile([P, RB], f32, tag="biasc")
            nc.gpsimd.dma_start(out=bias_sb[:], in_=bias_d[:])
            vt_sb = []
            for g in range(G):
                t = inp.tile([K, FD], fp8, tag=f"vt{g}", name=f"vt{g}_sb")
                nc.sync.dma_start(out=t[:], in_=vt_d[g][:])
                vt_sb.append(t)
            dmin_sb = stat.tile([P, RB * NCOL], f32)
            ones_t = stat.tile([P, WW], f32)
            nc.vector.memset(ones_t[:], 1.0)
            iota_t = stat.tile([P, WW], f32)
            nc.vector.tensor_tensor_scan(
                out=iota_t[:],
                data0=ones_t[:],
                data1=ones_t[:],
                initial=-1.0,
                op0=mybir.AluOpType.mult,
                op1=mybir.AluOpType.add,
            )
            mask_sb = stat.tile([P, RB * WW], bf16, tag="mask")
            for m in range(RB):
                nc.scalar.activation(
                    mask_sb[:, m * WW:(m + 1) * WW],
                    iota_t[:],
                    mybir.ActivationFunctionType.Sigmoid,
                    bias=bias_sb[:, m:m + 1],
                    scale=2.0e4,
                )

            for m in range(RB):
                gb = m // 2
                w_lo = WW * (m % 2)
                bw = w_lo + WW          # matmul'd cols in the boundary group
                for g in range(gb + 1):
                    ncols = FD if g < gb else bw
                    psum = psum_p.tile([P, FD], f32, tag="psum")
                    for j in range(ncols // MM_N):
                        in_window = g == gb and j * MM_N >= w_lo
                        nc.tensor.matmul(
                            psum[:, j * MM_N:(j + 1) * MM_N],
                            ur_sb[:, m * P:(m + 1) * P],
                            vt_sb[g][:, j * MM_N:(j + 1) * MM_N],
                            start=True,
                            stop=not in_window,
                        )
                        if in_window:
                            wcol = j * MM_N - w_lo
                            nc.tensor.matmul(
                                psum[:, j * MM_N:(j + 1) * MM_N],
                                id_sb[:],
                                mask_sb[:, m * WW + wcol:m * WW + wcol + MM_N],
                                start=False,
                                stop=True,
                            )
                    q = m * NCOL + g
                    if g < gb:
                        nc.vector.tensor_reduce(
                            dmin_sb[:, q:q + 1],
                            psum[:],
                            axis=mybir.AxisListType.X,
                            op=mybir.AluOpType.min,
                        )
                    else:
                        nc.vector.tensor_reduce(
                            dmin_sb[:, m * NCOL + G:m * NCOL + G + 1],
                            psum[:, 0:bw],
                            axis=mybir.AxisListType.X,
                            op=mybir.AluOpType.min,
                        )
            nc.sync.dma_start(out=dmin_d[:], in_=dmin_sb[:])
    nc.compile()
    return nc


_BUILDERS = {"l12": _build_l12, "l3": _build_l3}


def _get_program(name):
    if name not in _programs:
        _programs[name] = _BUILDERS[name]()
    return _programs[name]


TIMINGS = []  # (name, exec_time_ns) per launch, appended by _run


def _run(name, in_maps, trace=None):
    from concourse.bass_utils import run_bass_kernel_spmd

    if trace is None:
        trace = bool(int(os.environ.get("KERNEL_TRACE", "0")))
    nc = _get_program(name)
    res = run_bass_kernel_spmd(
        nc, in_maps, core_ids=list(range(NCORES)), trace=trace
    )
    TIMINGS.append((name, res.exec_time_ns))
    return res


def _augmented(data):
    """U (lhs rows) and V (rhs cols) of the K=68 augmented distance GEMM.

    fp8e4 operands with sq split into an fp8 hi+lo pair: d2 error ~1.2 abs
    (~1.4% at the dc^2 scale).  Every consumer decision has >=10x margin:
    count blur ~0.01 in dc^2, rho noise ~0.2% (the pipeline is entirely
    self-consistent in its own rho), delta vs threshold ~1000x.
    """
    import ml_dtypes

    f8 = ml_dtypes.float8_e4m3fn
    sq = np.einsum("ij,ij->i", data, data, dtype=np.float32).astype(np.float32)
    sqh = sq.astype(f8)
    sql = (sq - sqh.astype(np.float32)).astype(f8)
    ones = np.ones((N, 1), f8)
    zcol = lambda a: a[:, None]
    U = np.concatenate(
        [(-2.0 * data).astype(f8), zcol(sqh), zcol(sql), ones, ones], axis=1
    )
    V = np.concatenate(
        [data.astype(f8), ones, ones, zcol(sqh), zcol(sql)], axis=1
    )
    return U, V, sq


def _erf(x):
    """Abramowitz-Stegun 7.1.26 vectorized erf (|err| < 1.5e-7)."""
    s = np.sign(x)
    x = np.abs(x)
    t = 1.0 / (1.0 + 0.3275911 * x)
    y = 1.0 - (
        ((((1.061405429 * t - 1.453152027) * t) + 1.421413741) * t - 0.284496736)
        * t
        + 0.254829592
    ) * t * np.exp(-x * x)
    return s * y


def _phi(z):
    return 0.5 * (1.0 + _erf(z / np.sqrt(2.0)))


NGRID = 256


def _cv_corrections(sq):
    """Control-variate count corrections for the fixed count sample.

    Model P(d2 < t | sq_i, sq_j) ~ Phi((t - sq_i - sq_j)/(2 sqrt(sq_i sq_j/D)))
    and subtract the predicted row/col selection bias of the sampled
    rows/cols relative to the full point set.
    """
    sq64 = sq.astype(np.float64)
    step = N // NGRID
    grid = np.sort(sq64)[step // 2::step][:NGRID]

    def h(t, svals):
        s = svals[:, None]
        sp = grid[None, :]
        z = (t - s - sp) / (2.0 * np.sqrt(np.maximum(s * sp, 1e-9) / D))
        return _phi(z).mean(axis=1)

    dvec = np.zeros(NT)
    for b, (m, g) in enumerate(L1_GROUPS):
        t = float(DC2_GRID[b])
        h_all = h(t, grid).mean()
        d_row = h(t, sq64[m * P:(m + 1) * P]).mean() - h_all
        d_col = h(t, sq64[g * FD:g * FD + L1_W]).mean() - h_all
        dvec[b] = (d_row + d_col) * (P * L1_W)
    return dvec.astype(np.float32).reshape(1, NT)


def _host_fallback(data, rho_t, delta_t):
    """Pure-numpy reference path (only used if device assumptions break)."""
    data = np.asarray(data, np.float32)
    sq = np.sum(data * data, axis=1)
    d2 = sq[:, None] + sq[None, :] - 2.0 * (data @ data.T)
    dist = np.sqrt(np.maximum(d2, 0.0), dtype=np.float32)
    dc = np.percentile(dist, PCT)
    rho = np.exp(-((dist / dc) ** 2)).sum(axis=1).astype(np.float32)
    higher = rho[None, :] > rho[:, None]
    masked = np.where(higher, dist, np.inf)
    delta_m = masked.min(axis=1)
    nhd_m = masked.argmin(axis=1)
    has = higher.any(axis=1)
    delta = np.where(has, delta_m, dist.max(axis=1))
    nhd = np.where(has, nhd_m, np.arange(N))
    return _finish_labels(rho, delta, nhd, rho_t, delta_t)


def _finish_labels(rho, delta, nhd, rho_t, delta_t):
    is_center = (rho > rho_t) & (delta > delta_t)
    center_rank = np.cumsum(is_center.astype(np.int32)) - 1
    labels = np.where(is_center, center_rank, -1).astype(np.int32)
    order = np.argsort(-rho, kind="stable")
    for i in order:
        if labels[i] < 0:
            labels[i] = labels[nhd[i]]
    return labels


def kernel(data, rho_threshold, delta_threshold):
    data = np.ascontiguousarray(np.asarray(data, dtype=np.float32))
    assert data.shape == (N, D)
    rho_t = float(np.asarray(rho_threshold))
    delta_t = float(np.asarray(delta_threshold))

    U, V, sq = _augmented(data)
    VT = np.ascontiguousarray(V.T)  # [K, N]

    # ---- L12: counts -> on-device dc^2 -> rho (single launch) ----------
    tvec = DC2_GRID.astype(np.float32).reshape(1, NT)
    dvec_sig = _cv_corrections(sq).astype(np.float64)
    dvec = (2.0 * dvec_sig - float(P * L1_W)).astype(np.float32)
    thr = np.broadcast_to(
        (0.5 * SIG_ALPHA * DC2_GRID).astype(np.float32)[None, :], (P, NT)
    ).copy()
    uvc = np.ascontiguousarray(U[0:ROWS].T)
    ws = [VT[:, g * FD:g * FD + L1_W] for b, (m, g) in enumerate(L1_GROUPS)]
    h1 = np.ascontiguousarray(np.concatenate([uvc, ws[0]], axis=1))
    h2 = np.ascontiguousarray(np.concatenate(ws[1:], axis=1))
    vt_in = {
        f"vt{g}": np.ascontiguousarray(VT[:, g * FD:(g + 1) * FD])
        for g in range(G)
    }
    in_maps = [
        {
            "h1": h1,
            "h2": h2,
            "thr": thr,
            "ur": np.ascontiguousarray(U[c * ROWS:(c + 1) * ROWS].T),
            "tvec": tvec,
            "dvec": dvec,
            **vt_in,
        }
        for c in range(NCORES)
    ]
    r12 = _run("l12", in_maps)

    # validate the on-device dc interpolation (tanh units: q = 2*(c - dvec_sig))
    q = r12.results[0]["counts"].astype(np.float64).sum(axis=0) - dvec[0].astype(
        np.float64
    )
    brackets = [b for b in range(NT - 1) if q[b] <= CSTAR2 < q[b + 1]]
    if len(brackets) != 1 or not np.all(np.diff(q) > 0):
        return _host_fallback(data, rho_t, delta_t)

    rho = np.empty(N, np.float32)
    for c in range(NCORES):
        out = r12.results[c]["rho"]  # [P, RB]
        rho[c * ROWS:(c + 1) * ROWS] = out.T.reshape(-1)
    if not np.all(np.isfinite(rho)) or rho.min() < 0.5 or rho.max() > N + 1:
        return _host_fallback(data, rho_t, delta_t)

    # ---- host: sort by rho desc; prefix cutoffs ------------------------
    order = np.argsort(-rho, kind="stable")
    rho_sorted = rho[order]
    # c_i = #points with rho strictly greater (ties excluded)
    cuts = np.searchsorted(-rho_sorted, -rho_sorted, side="left").astype(np.int64)

    data_p = data[order]
    sq_p = sq[order]
    Up = U[order]
    Vp = V[order]
    rhs_p = np.ascontiguousarray(Vp.T)

    # round-robin block interleave: core c <- sorted blocks 8m + c
    NB = N // P  # 64 sorted row-blocks
    blk_rows = np.arange(N).reshape(NB, P)
    core_rows = [blk_rows[np.arange(RB) * NCORES + c].reshape(-1) for c in range(NCORES)]

    import ml_dtypes

    bf = ml_dtypes.bfloat16
    idb = (MASK_BIG * np.eye(P, dtype=np.float32)).astype(bf)
    vt3_in = {
        f"vt{g}": np.ascontiguousarray(rhs_p[:, g * FD:(g + 1) * FD])
        for g in range(G)
    }
    in_maps = []
    for c in range(NCORES):
        rows = core_rows[c]
        biasc = np.empty((P, RB), np.float32)
        for m in range(RB):
            base = (m // 2) * FD + WW * (m % 2)
            cutrel = np.clip(cuts[rows[m * P:(m + 1) * P]] - base, 0, WW)
            biasc[:, m] = 2.0e4 * (0.5 - cutrel.astype(np.float64))
        in_maps.append(
            {
                "ur": np.ascontiguousarray(Up[rows].T),
                "idb": idb,
                "biasc": biasc,
                **vt3_in,
            }
        )
    r3 = _run("l3", in_maps)
    # dmin[i] holds per-source minima; slot k < gb = full group k,
    # slot G = boundary group (prefix + masked window in one reduce)
    dmin = np.full((N, NCOL), np.inf, np.float32)
    for c in range(NCORES):
        out = r3.results[c]["dmin"]  # [P, RB*NCOL]
        rows = core_rows[c]
        for m in range(RB):
            gb = m // 2
            blk = rows[m * P:(m + 1) * P]
            for g in range(gb):
                dmin[blk, g] = out[:, m * NCOL + g]
            dmin[blk, G] = out[:, m * NCOL + G]

    # ---- host: delta, fallback rows, centers, nhd (lazy), labels -------
    delta2_sorted = dmin.min(axis=1)

    # rho-tie rows whose cutoff dips below their block's boundary window:
    # the device's unmasked prefix included a few extra columns; fix exactly.
    win_base = ((np.arange(N) // P) // NCORES) * WW  # 1024*m per sorted row
    straddle_fix = {}
    for i in np.nonzero(cuts < win_base)[0]:
        cut = int(cuts[i])
        if cut == 0:
            delta2_sorted[i] = np.inf
            continue
        d2row = sq_p[i] + sq_p[:cut] - 2.0 * (data_p[:cut] @ data_p[i])
        j = int(np.argmin(d2row))
        delta2_sorted[i] = d2row[j]
        straddle_fix[i] = j

    empty = delta2_sorted >= EMPTY_SENTINEL  # no higher-density point
    delta_sorted = np.sqrt(np.maximum(delta2_sorted, 0.0), dtype=np.float32)
    for i in np.nonzero(empty)[0]:
        d2row = sq_p[i] + sq_p - 2.0 * (data_p @ data_p[i])
        delta_sorted[i] = np.sqrt(max(float(np.max(np.maximum(d2row, 0.0))), 0.0))

    delta = np.empty(N, np.float32)
    delta[order] = delta_sorted

    is_center = (rho > rho_t) & (delta > delta_t)
    center_rank = np.cumsum(is_center.astype(np.int32)) - 1
    labels = np.where(is_center, center_rank, -1).astype(np.int32)

    need_nhd = ~is_center[order]  # sorted positions whose label must propagate
    nhd = np.arange(N, dtype=np.int64)  # default: self (matches reference)
    for i in np.nonzero(need_nhd)[0]:
        if empty[i]:
            continue  # nhd stays self, as in reference
        if i in straddle_fix:
            nhd[order[i]] = order[straddle_fix[i]]
            continue
        k = int(np.argmin(dmin[i]))
        m = (i // P) // NCORES
        gb = m // 2
        w_lo = WW * (m % 2)
        if k == G:
            c0, clen = gb * FD, w_lo + WW
        else:
            c0, clen = k * FD, FD
        end_local = int(np.clip(cuts[i] - c0, 0, clen))
        cols = slice(c0, c0 + end_local)
        d2part = sq_p[i] + sq_p[cols] - 2.0 * (data_p[cols] @ data_p[i])
        j_local = int(np.argmin(d2part))
        nhd[order[i]] = order[c0 + j_local]

    for i in order:
        if labels[i] < 0:
            labels[i] = labels[nhd[i]]
    return labels.astype(np.int32)
